# revision 1
# baseline (speedup 1.0000x reference)
"""Batchelor GPU-NUFFT forward operator on 8 Trainium2 NeuronCores.

Math (per timepoint t):
    warped  = bilinear_warp(image, flow[..., t])
    coil    = csm * warped                                  [Nc,Nx,Ny]
    out_t[c,s] = sum_{x,y} coil[c,x,y] exp(-2pi i (kx_s (x-64) + ky_s (y-64)))
    out     = sum_t out_t                                   [Nc,NS] complex64

Sharding: 8 cores = 4 timepoints x 2 sample-halves (4096 samples each).
Host unshard: sum the 4 timepoint partials per half, concat halves.

Per-core device algorithm:
  * warp: bf16 corner table (8 corner values per pixel, padded to the 256B
    SWDGE element minimum) fetched with 16 dma_gather ops of 1024 indices
    each (the HW descriptor ring caps at ~128 descriptors/instruction; one
    descriptor covers 16 indices). Placement contract: gathered element i
    lands at out[i%128, i//128], indices sit int16-wrapped at
    idx[i%16, i//16]. Slot i = x*128 + y makes consecutive descriptors read
    consecutive table rows (DRAM-sequential). The warp runs in [y, x]
    layout; the two warped planes are transposed back with a PE transpose.
  * NUFFT: Khatri-Rao split y = yo*8 + yi; per 512-sample chunk 32
    accumulating bf16 matmuls (stationary = packed coil, moving = cos/sin
    tiles) into FOUR psum banks (Cr cos, Cr sin, Ci cos, Ci sin) so no
    negated sin tile is needed; the outer phase factor is applied as eight
    single-PSUM-operand products on DVE, and the +- combination plus the
    yo-reduction fold into signed 0/1 selector matmuls on the PE.
    Trig args are range-reduced once by the +-1.5*2^23 round trick, then
    per-yi by one add_range_wrap custom DVE op; cos(2pi m) uses Abs+bias on
    the Scalar engine for the first NABS yi and sin(2pi wrap(m+1/4)) on DVE
    for the rest (a DVE<->Scalar balance knob). Trig for chunk ch+1 is
    emitted before the matmuls of chunk ch; the first chunk's trig runs
    during the gather.
"""

import sys

if "/opt/trn_rl_repo" not in sys.path:
    sys.path.insert(0, "/opt/trn_rl_repo")

import math

import numpy as np

import concourse.bass as bass
import concourse.tile as tile
from concourse import bacc
from concourse import mybir

P = 128
NX = 128
NCOIL = 8
NS = 8192
NT = 4
S = 4096  # samples per core (half of NS)
CH = 512  # samples per inner chunk
NCHUNK = S // CH
YI = 8
YO = 16
NPIX = NX * NX
NGATH = 16
GIDX = NPIX // NGATH  # 1024 indices per gather
ELEM = 128  # bf16 elements per table row = 256 bytes
NABS = 6  # yi < NABS use the Scalar Abs cos path; rest the DVE wrap path
LEAD = 2  # chunks of trig emitted ahead of the gather-dependent ops
KBUF = 3

F32 = mybir.dt.float32
BF16 = mybir.dt.bfloat16
F32R = mybir.dt.float32r
I16 = mybir.dt.int16
TWO_PI = float(2.0 * math.pi)
MAGIC = 12582912.0  # 1.5*2^23: (x + M) - M == round-to-nearest(x)
ALU = mybir.AluOpType
ACTF = mybir.ActivationFunctionType


def build_program(nc: bass.Bass, dbg: bool = False):
    def dbg_out(name, src_ap, shape, dtype=F32):
        if not dbg:
            return
        d = nc.dram_tensor("dbg_" + name, shape, dtype, kind="ExternalOutput").ap()
        nc.sync.dma_start(d[:], src_ap)

    image_r = nc.dram_tensor("image_r", [NX, NX], F32, kind="ExternalInput").ap()
    image_i = nc.dram_tensor("image_i", [NX, NX], F32, kind="ExternalInput").ap()
    csm_r = nc.dram_tensor("csm_r", [NCOIL, NX, NX], F32, kind="ExternalInput").ap()
    csm_i = nc.dram_tensor("csm_i", [NCOIL, NX, NX], F32, kind="ExternalInput").ap()
    kx_d = nc.dram_tensor("kx", [S], F32, kind="ExternalInput").ap()
    ky_d = nc.dram_tensor("ky", [S], F32, kind="ExternalInput").ap()
    # pre-added warp coordinates: cxt/cyt in transposed [y, x] layout,
    # cxw/cyw in the gather's wrapped layout
    # ([p, s] -> pixel (x = s//8, y = 16*(s%8) + p%16), replicated 8x).
    cxt_d = nc.dram_tensor("cxt", [NX, NX], F32, kind="ExternalInput").ap()
    cyt_d = nc.dram_tensor("cyt", [NX, NX], F32, kind="ExternalInput").ap()
    cxw_d = nc.dram_tensor("cxw", [P, GIDX], F32, kind="ExternalInput").ap()
    cyw_d = nc.dram_tensor("cyw", [P, GIDX], F32, kind="ExternalInput").ap()
    out_r = nc.dram_tensor("out_r", [NCOIL, S], F32, kind="ExternalOutput").ap()
    out_i = nc.dram_tensor("out_i", [NCOIL, S], F32, kind="ExternalOutput").ap()
    img64_d = nc.dram_tensor("img64_scratch", [NPIX, ELEM], BF16,
                             kind="Internal").ap()

    # ---------------- inline constants ----------------
    pvals = np.arange(P, dtype=np.float32)
    xc_d = nc.inline_tensor((pvals - 64.0).reshape(P, 1), name="c_xc").ap()
    yo8_d = nc.inline_tensor((8.0 * (np.arange(P) % 16)).astype(np.float32)
                             .reshape(P, 1), name="c_yo8").ap()
    half_pi_d = nc.inline_tensor(np.full((P, 1), math.pi / 2, np.float32),
                                 name="c_half_pi").ap()
    ident_d = nc.inline_tensor(np.eye(P, dtype=np.float32), name="c_ident").ap()
    sel_np = (np.arange(P)[:, None] // 16 == np.arange(NCOIL)[None, :]).astype(
        np.float32)
    selpm_np = np.concatenate([sel_np, -sel_np], axis=1)  # [128, 16]: +sel | -sel
    selpm_d = nc.inline_tensor(selpm_np, name="c_selpm").ap()

    with tile.TileContext(nc) as tc, \
         tc.tile_pool(name="pp", bufs=1) as pp, \
         tc.tile_pool(name="gp", bufs=1) as gp:

        xc_col = pp.tile([P, 1], F32)
        nc.sync.dma_start(xc_col[:], xc_d[:])
        yo8 = pp.tile([P, 1], F32)
        nc.sync.dma_start(yo8[:], yo8_d[:])
        half_pi = pp.tile([P, 1], F32)
        nc.sync.dma_start(half_pi[:], half_pi_d[:])
        ident = pp.tile([P, P], F32)
        nc.sync.dma_start(ident[:], ident_d[:])
        selpm32 = pp.tile([P, 2 * NCOIL], F32)
        nc.sync.dma_start(selpm32[:], selpm_d[:])
        selpm = pp.tile([P, 2 * NCOIL], BF16)
        nc.vector.tensor_copy(selpm[:], selpm32[:])

        # persistent packed coil stationary (bf16); outlives the warp pools
        RA = pp.tile([P, YI, 256], BF16)
        # gather output + its index tile live until the pack completes
        g8p = gp.tile([P, NX, ELEM], BF16)
        idx16 = gp.tile([P, GIDX], I16)

        trig = {}
        kbc = {}

        def emit_kbc(ch):
            kxc = pp.tile([P, CH], F32, tag="kxc", bufs=2)
            nc.sync.dma_start(
                kxc[:], kx_d[ch * CH:(ch + 1) * CH]
                .rearrange("(p s) -> p s", p=1).to_broadcast([P, CH]))
            kyc = pp.tile([P, CH], F32, tag="kyc", bufs=2)
            nc.sync.dma_start(
                kyc[:], ky_d[ch * CH:(ch + 1) * CH]
                .rearrange("(p s) -> p s", p=1).to_broadcast([P, CH]))
            kbc[ch] = (kxc, kyc)

        def emit_trig(ch):
            if ch not in kbc:
                emit_kbc(ch)
            kxc, kyc = kbc.pop(ch)
            u = lp.tile([P, CH], F32, tag="u", bufs=2)
            nc.scalar.mul(u[:], kxc[:], xc_col[:, 0:1])
            a0 = lp.tile([P, CH], F32, tag="a0", bufs=2)
            nc.vector.scalar_tensor_tensor(a0[:], kyc[:], -64.0, u[:],
                                           op0=ALU.mult, op1=ALU.add)
            r0 = lp.tile([P, CH], F32, tag="r0", bufs=1)
            nc.vector.tensor_scalar(r0[:], a0[:], MAGIC, MAGIC,
                                    op0=ALU.add, op1=ALU.subtract)
            ta = lp.tile([P, CH], F32, tag="ta", bufs=2)
            nc.scalar.mul(ta[:], kyc[:], yo8[:, 0:1])
            ra = lp.tile([P, CH], F32, tag="ra", bufs=1)
            nc.vector.tensor_scalar(ra[:], ta[:], MAGIC, MAGIC,
                                    op0=ALU.add, op1=ALU.subtract)
            m2a = lp.tile([P, CH], F32, tag="m2a", bufs=1)
            nc.vector.tensor_tensor(m2a[:], ta[:], ra[:], op=ALU.subtract)
            mca = lp.tile([P, CH], F32, tag="mca", bufs=1)
            nc.vector.add_range_wrap(mca[:], m2a[:], 0.25, 0.5, 1.0)
            aic = lp.tile([P, CH], F32, tag="aic", bufs=KBUF)
            nc.scalar.activation(aic[:], m2a[:], ACTF.Sin, scale=-TWO_PI)
            arc = lp.tile([P, CH], F32, tag="arc", bufs=KBUF)
            nc.scalar.activation(arc[:], mca[:], ACTF.Sin, scale=TWO_PI)

            kits, krts = [], []
            prev_m2 = None
            for yi in range(YI):
                m2 = lp.tile([P, CH], F32, tag="m2", bufs=3)
                if yi == 0:
                    nc.vector.tensor_tensor(m2[:], a0[:], r0[:], op=ALU.subtract)
                else:
                    s_t = lp.tile([P, CH], F32, tag="s_t", bufs=2)
                    nc.gpsimd.tensor_tensor(s_t[:], prev_m2[:], kyc[:], op=ALU.add)
                    nc.vector.add_range_wrap(m2[:], s_t[:], 0.0, 0.5, 1.0)
                kit = kp.tile([P, CH], BF16, tag=f"kit{yi}", bufs=KBUF)
                nc.scalar.activation(kit[:], m2[:], ACTF.Sin, scale=-TWO_PI)
                krt = kp.tile([P, CH], BF16, tag=f"krt{yi}", bufs=KBUF)
                if yi < NABS:
                    mabs = lp.tile([P, CH], F32, tag="mabs", bufs=1)
                    nc.scalar.activation(mabs[:], m2[:], ACTF.Abs)
                    nc.scalar.activation(krt[:], mabs[:], ACTF.Sin,
                                         scale=-TWO_PI, bias=half_pi[:, 0:1])
                else:
                    mc = lp.tile([P, CH], F32, tag="mc", bufs=3)
                    nc.vector.add_range_wrap(mc[:], m2[:], 0.25, 0.5, 1.0)
                    nc.scalar.activation(krt[:], mc[:], ACTF.Sin, scale=TWO_PI)
                kits.append(kit)
                krts.append(krt)
                prev_m2 = m2
            trig[ch] = (kits, krts, arc, aic)

        # ================ warp phase A: indices + table + gathers ============
        with tc.tile_pool(name="wpa", bufs=1) as wa:
            cxw = wa.tile([P, GIDX], F32)
            nc.sync.dma_start(cxw[:], cxw_d[:])
            cyw = wa.tile([P, GIDX], F32)
            nc.sync.dma_start(cyw[:], cyw_d[:])

            wt0 = wa.tile([P, GIDX], F32, tag="wt", bufs=3)
            nc.vector.tensor_scalar(wt0[:], cxw[:], 127.0, 0.0,
                                    op0=ALU.min, op1=ALU.max)
            wt1 = wa.tile([P, GIDX], F32, tag="wt", bufs=3)
            nc.vector.tensor_scalar(wt1[:], wt0[:], 0.5, None, op0=ALU.subtract)
            x0w = wa.tile([P, GIDX], F32)
            nc.vector.tensor_scalar(x0w[:], wt1[:], MAGIC, MAGIC,
                                    op0=ALU.add, op1=ALU.subtract)
            wt2 = wa.tile([P, GIDX], F32, tag="wt", bufs=3)
            nc.vector.tensor_scalar(wt2[:], cyw[:], 127.0, 0.0,
                                    op0=ALU.min, op1=ALU.max)
            wt3 = wa.tile([P, GIDX], F32, tag="wt", bufs=3)
            nc.vector.tensor_scalar(wt3[:], wt2[:], 0.5, None, op0=ALU.subtract)
            y0w = wa.tile([P, GIDX], F32)
            nc.vector.tensor_scalar(y0w[:], wt3[:], MAGIC, MAGIC,
                                    op0=ALU.add, op1=ALU.subtract)
            wt4 = wa.tile([P, GIDX], F32, tag="wt", bufs=3)
            nc.vector.tensor_scalar(wt4[:], x0w[:], 128.0, None, op0=ALU.mult)
            wt5 = wa.tile([P, GIDX], F32, tag="wt", bufs=3)
            nc.vector.tensor_tensor(wt5[:], wt4[:], y0w[:], op=ALU.add)
            nc.vector.tensor_copy(idx16[:], wt5[:])
            dbg_out("idx16", idx16[:], [P, GIDX], I16)

            # corner table, normal [x, y] layout, bf16
            img_r_sb = wa.tile([P, NX], F32)
            nc.sync.dma_start(img_r_sb[:], image_r[:])
            img_i_sb = wa.tile([P, NX], F32)
            nc.sync.dma_start(img_i_sb[:], image_i[:])
            imgBr = wa.tile([P, NX], F32)
            nc.sync.dma_start(imgBr[0:127, :], img_r_sb[1:128, :])
            nc.sync.dma_start(imgBr[127:128, :], img_r_sb[127:128, :])
            imgBi = wa.tile([P, NX], F32)
            nc.sync.dma_start(imgBi[0:127, :], img_i_sb[1:128, :])
            nc.sync.dma_start(imgBi[127:128, :], img_i_sb[127:128, :])

            img8 = wa.tile([P, NX, ELEM], BF16)
            for k, s_ in ((0, img_r_sb), (2, imgBr), (4, img_i_sb), (6, imgBi)):
                nc.scalar.copy(img8[:, :, k], s_[:])
                nc.scalar.copy(img8[:, 0:127, k + 1], s_[:, 1:128])
                nc.scalar.copy(img8[:, 127:128, k + 1], s_[:, 127:128])
            nc.sync.dma_start(
                img64_d.rearrange("(x y) k -> x (y k)", x=NX),
                img8[:].rearrange("p y k -> p (y k)"))

            gsems = [nc.alloc_semaphore(f"gath_sem{q}") for q in range(4)]
            for h in range(NGATH):
                nc.gpsimd.dma_gather(
                    out_ap=g8p[:, h * 8:(h + 1) * 8, :],
                    in_ap=img64_d[:],
                    idxs_ap=idx16[:, h * 64:(h + 1) * 64],
                    num_idxs=GIDX,
                    num_idxs_reg=GIDX,
                    elem_size=ELEM,
                    queue_num=h % 4,
                ).then_inc(gsems[h % 4], 16)

        def emit_mm(ch):
            c0 = ch * CH
            kits, krts, arc, aic = trig.pop(ch)
            p1 = ps.tile([P, CH], F32, tag="p1")
            p2 = ps.tile([P, CH], F32, tag="p2")
            p3 = ps.tile([P, CH], F32, tag="p3")
            p4 = ps.tile([P, CH], F32, tag="p4")
            for yi in range(YI):
                st, sp = (yi == 0), (yi == YI - 1)
                nc.tensor.matmul(p1[:], RA[:, yi, 0:128], krts[yi][:],
                                 start=st, stop=sp)
                nc.tensor.matmul(p2[:], RA[:, yi, 0:128], kits[yi][:],
                                 start=st, stop=sp)
                nc.tensor.matmul(p3[:], RA[:, yi, 128:256], krts[yi][:],
                                 start=st, stop=sp)
                nc.tensor.matmul(p4[:], RA[:, yi, 128:256], kits[yi][:],
                                 start=st, stop=sp)

            # outer phase products (one PSUM operand each), bf16 out
            q1 = lp.tile([P, CH], BF16, tag="q1", bufs=2)
            nc.vector.tensor_tensor(q1[:], p1[:], arc[:], op=ALU.mult)
            q2 = lp.tile([P, CH], BF16, tag="q2", bufs=2)
            nc.vector.tensor_tensor(q2[:], p2[:], aic[:], op=ALU.mult)
            q3 = lp.tile([P, CH], BF16, tag="q3", bufs=2)
            nc.vector.tensor_tensor(q3[:], p3[:], aic[:], op=ALU.mult)
            q4 = lp.tile([P, CH], BF16, tag="q4", bufs=2)
            nc.vector.tensor_tensor(q4[:], p4[:], arc[:], op=ALU.mult)
            w1 = lp.tile([P, CH], BF16, tag="w1", bufs=2)
            nc.vector.tensor_tensor(w1[:], p2[:], arc[:], op=ALU.mult)
            w2 = lp.tile([P, CH], BF16, tag="w2", bufs=2)
            nc.vector.tensor_tensor(w2[:], p3[:], arc[:], op=ALU.mult)
            w3 = lp.tile([P, CH], BF16, tag="w3", bufs=2)
            nc.vector.tensor_tensor(w3[:], p1[:], aic[:], op=ALU.mult)
            w4q = lp.tile([P, CH], BF16, tag="w4q", bufs=2)
            nc.vector.tensor_tensor(w4q[:], p4[:], aic[:], op=ALU.mult)

            # out_r = sel(q1) - sel(q2+q3+q4); out_i = sel(w1+w2+w3) - sel(w4q)
            qs1 = lp.tile([P, CH], BF16, tag="qs1", bufs=2)
            nc.gpsimd.tensor_tensor(qs1[:], q2[:], q3[:], op=ALU.add)
            qs2 = lp.tile([P, CH], BF16, tag="qs2", bufs=2)
            nc.gpsimd.tensor_tensor(qs2[:], qs1[:], q4[:], op=ALU.add)
            ws1 = lp.tile([P, CH], BF16, tag="ws1", bufs=2)
            nc.gpsimd.tensor_tensor(ws1[:], w1[:], w2[:], op=ALU.add)
            ws2 = lp.tile([P, CH], BF16, tag="ws2", bufs=2)
            nc.gpsimd.tensor_tensor(ws2[:], ws1[:], w3[:], op=ALU.add)
            SP, SM = selpm[:, 0:NCOIL], selpm[:, NCOIL:2 * NCOIL]
            orps = pso.tile([NCOIL, CH], F32, tag="or")
            nc.tensor.matmul(orps[:], SP, q1[:], start=True, stop=False)
            nc.tensor.matmul(orps[:], SM, qs2[:], start=False, stop=True)
            oips = pso.tile([NCOIL, CH], F32, tag="oi")
            nc.tensor.matmul(oips[:], SP, ws2[:], start=True, stop=False)
            nc.tensor.matmul(oips[:], SM, w4q[:], start=False, stop=True)

            osr = lp.tile([NCOIL, CH], F32, tag="osr", bufs=1)
            nc.vector.tensor_copy(osr[:], orps[:])
            osi = lp.tile([NCOIL, CH], F32, tag="osi", bufs=1)
            nc.vector.tensor_copy(osi[:], oips[:])
            nc.sync.dma_start(out_r[:, c0:c0 + CH], osr[:])
            nc.sync.dma_start(out_i[:, c0:c0 + CH], osi[:])

        # ================ main-loop pools ================
        lp_ctx = tc.tile_pool(name="loop", bufs=1)
        lp = lp_ctx.__enter__()
        kp_ctx = tc.tile_pool(name="kr", bufs=1)
        kp = kp_ctx.__enter__()
        ps_ctx = tc.tile_pool(name="ps", bufs=1, space="PSUM")
        ps = ps_ctx.__enter__()
        pso_ctx = tc.tile_pool(name="pso", bufs=1, space="PSUM")
        pso = pso_ctx.__enter__()

        # trig for the first LEAD chunks runs while the gather is in flight
        for ch in range(LEAD):
            emit_trig(ch)

        # ============ warp phase B: weights + combine + coil + pack ==========
        with tc.tile_pool(name="wpb", bufs=1) as wb:
            fl0t = wb.tile([P, NX], F32)
            nc.sync.dma_start(fl0t[:], cxt_d[:])
            fl1t = wb.tile([P, NX], F32)
            nc.sync.dma_start(fl1t[:], cyt_d[:])
            cx2 = wb.tile([P, NX], F32)
            nc.vector.tensor_scalar(cx2[:], fl0t[:], 127.0, 0.0,
                                    op0=ALU.min, op1=ALU.max)
            cy2 = wb.tile([P, NX], F32)
            nc.vector.tensor_scalar(cy2[:], fl1t[:], 127.0, 0.0,
                                    op0=ALU.min, op1=ALU.max)
            c5x = wb.tile([P, NX], F32, tag="bt", bufs=3)
            nc.vector.tensor_scalar(c5x[:], cx2[:], 0.5, None, op0=ALU.subtract)
            x0 = wb.tile([P, NX], F32)
            nc.vector.tensor_scalar(x0[:], c5x[:], MAGIC, MAGIC,
                                    op0=ALU.add, op1=ALU.subtract)
            wx = wb.tile([P, NX], F32)
            nc.vector.tensor_tensor(wx[:], cx2[:], x0[:], op=ALU.subtract)
            c5y = wb.tile([P, NX], F32, tag="bt", bufs=3)
            nc.vector.tensor_scalar(c5y[:], cy2[:], 0.5, None, op0=ALU.subtract)
            y0 = wb.tile([P, NX], F32)
            nc.vector.tensor_scalar(y0[:], c5y[:], MAGIC, MAGIC,
                                    op0=ALU.add, op1=ALU.subtract)
            wy = wb.tile([P, NX], F32)
            nc.vector.tensor_tensor(wy[:], cy2[:], y0[:], op=ALU.subtract)
            onemwx = wb.tile([P, NX], F32)
            nc.vector.tensor_scalar(onemwx[:], wx[:], -1.0, 1.0,
                                    op0=ALU.mult, op1=ALU.add)
            onemwy = wb.tile([P, NX], F32)
            nc.vector.tensor_scalar(onemwy[:], wy[:], -1.0, 1.0,
                                    op0=ALU.mult, op1=ALU.add)
            w4 = wb.tile([P, NX, 4], F32)
            nc.vector.tensor_tensor(w4[:, :, 0], onemwx[:], onemwy[:], op=ALU.mult)
            nc.vector.tensor_tensor(w4[:, :, 1], onemwx[:], wy[:], op=ALU.mult)
            nc.vector.tensor_tensor(w4[:, :, 2], wx[:], onemwy[:], op=ALU.mult)
            nc.vector.tensor_tensor(w4[:, :, 3], wx[:], wy[:], op=ALU.mult)

            # bilinear combine in [y, x] layout (g8p strided bf16 corners)
            for q in range(4):
                nc.vector.wait_ge(gsems[q], 16 * (NGATH // 4))
            t8r = wb.tile([P, NX, 4], F32, tag="t8", bufs=1)
            nc.vector.tensor_tensor(t8r[:], g8p[:, :, 0:4], w4[:], op=ALU.mult)
            warped_rt = wb.tile([P, NX], F32)
            nc.vector.reduce_sum(warped_rt[:], t8r[:], axis=mybir.AxisListType.X)
            t8i = wb.tile([P, NX, 4], F32, tag="t8", bufs=1)
            nc.vector.tensor_tensor(t8i[:], g8p[:, :, 4:8], w4[:], op=ALU.mult)
            warped_it = wb.tile([P, NX], F32)
            nc.vector.reduce_sum(warped_it[:], t8i[:], axis=mybir.AxisListType.X)

            trp = ps.tile([P, NX], F32, tag="trp")
            nc.tensor.transpose(trp[:], warped_rt[:], ident[:])
            warped_r = wb.tile([P, NX], F32)
            nc.scalar.copy(warped_r[:], trp[:])
            tip = ps.tile([P, NX], F32, tag="tip")
            nc.tensor.transpose(tip[:], warped_it[:], ident[:])
            warped_i = wb.tile([P, NX], F32)
            nc.scalar.copy(warped_i[:], tip[:])
            dbg_out("warped_r", warped_r[:], [P, NX])
            dbg_out("warped_i", warped_i[:], [P, NX])

            # coil = csm * warped, packed bf16
            csm_r_sb = wb.tile([P, NCOIL, NX], F32)
            nc.sync.dma_start(csm_r_sb[:], csm_r.rearrange("c x y -> x c y"))
            csm_i_sb = wb.tile([P, NCOIL, NX], F32)
            nc.sync.dma_start(csm_i_sb[:], csm_i.rearrange("c x y -> x c y"))

            wr_b = warped_r[:].rearrange("p (c y) -> p c y", c=1).to_broadcast(
                [P, NCOIL, NX])
            wi_b = warped_i[:].rearrange("p (c y) -> p c y", c=1).to_broadcast(
                [P, NCOIL, NX])

            tt1 = wb.tile([P, NCOIL, NX], F32, tag="tta", bufs=1)
            nc.vector.tensor_tensor(tt1[:], csm_r_sb[:], wr_b, op=ALU.mult)
            tt2 = wb.tile([P, NCOIL, NX], F32, tag="ttb", bufs=1)
            nc.vector.tensor_tensor(tt2[:], csm_i_sb[:], wi_b, op=ALU.mult)

            def pack_view(r):
                return RA[:].rearrange(
                    "p yi (r c yo) -> p r c (yo yi)", r=2, c=NCOIL)[:, r] \
                    .rearrange("p c (yo yi) -> p c yo yi", yi=YI) \
                    .rearrange("p c yo yi -> p c (yo yi)")

            def pack_dst(r):
                # RA[:, yi, r*128 + c*16 + yo] <- coil[p, c, yo*8+yi]
                return RA[:].rearrange("p yi (r c yo) -> p r c yo yi",
                                       r=2, c=NCOIL)[:, r]

            def coil_as(t):
                return t.rearrange("p c (yo yi) -> p c yo yi", yi=YI)

            crt = wb.tile([P, NCOIL, NX], F32, tag="crt", bufs=1)
            nc.vector.tensor_tensor(crt[:], tt1[:], tt2[:], op=ALU.subtract)
            nc.vector.tensor_copy(pack_dst(0), coil_as(crt[:]))
            tt3 = wb.tile([P, NCOIL, NX], F32, tag="tta", bufs=1)
            nc.vector.tensor_tensor(tt3[:], csm_r_sb[:], wi_b, op=ALU.mult)
            tt4 = wb.tile([P, NCOIL, NX], F32, tag="ttb", bufs=1)
            nc.vector.tensor_tensor(tt4[:], csm_i_sb[:], wr_b, op=ALU.mult)
            cit = wb.tile([P, NCOIL, NX], F32, tag="crt", bufs=1)
            nc.vector.tensor_tensor(cit[:], tt3[:], tt4[:], op=ALU.add)
            nc.vector.tensor_copy(pack_dst(1), coil_as(cit[:]))

        # ---- interleaved matmul/trig emission ----
        for ch in range(NCHUNK):
            nxt = ch + LEAD
            if nxt < NCHUNK:
                emit_trig(nxt)
            emit_mm(ch)

        pso_ctx.__exit__(None, None, None)
        ps_ctx.__exit__(None, None, None)
        kp_ctx.__exit__(None, None, None)
        lp_ctx.__exit__(None, None, None)


_COMPILED = {}


def _get_nc(dbg: bool = False):
    key = ("nc", dbg)
    if key not in _COMPILED:
        nc = bacc.Bacc("TRN2", debug=False, num_swdge_queues=4)
        build_program(nc, dbg=dbg)
        nc.compile()
        _COMPILED[key] = nc
    return _COMPILED[key]


# wrapped-layout grids: position (p, s) <-> pixel (x = s//8, y = 16*(s%8) + p%16)
_SW = np.arange(GIDX)
_XGW = np.tile(_SW // 8, (P, 1)).astype(np.int64)
_YGW = ((_SW[None, :] % 8) * 16 + (np.arange(P)[:, None] % 16)).astype(np.int64)
_JGRID = np.arange(NX, dtype=np.float32)


def make_in_maps(image_r, image_i, csm_r, csm_i, traj, dcf, flow):
    del dcf  # unused by the operator
    in_maps = []
    for core in range(8):
        t, h = divmod(core, 2)
        sl = slice(h * S, (h + 1) * S)
        f0 = np.ascontiguousarray(flow[:, :, 0, t], np.float32)
        f1 = np.ascontiguousarray(flow[:, :, 1, t], np.float32)
        # pre-added coordinates (grid + displacement)
        cxt = f0.T + _JGRID[None, :]          # [y, x]: x-coordinate
        cyt = f1.T + _JGRID[:, None]          # [y, x]: y-coordinate (row = y)
        cxw = (f0 + _JGRID[:, None])[_XGW, _YGW]
        cyw = (f1 + _JGRID[None, :])[_XGW, _YGW]
        in_maps.append({
            "image_r": np.ascontiguousarray(image_r, np.float32),
            "image_i": np.ascontiguousarray(image_i, np.float32),
            "csm_r": np.ascontiguousarray(csm_r, np.float32),
            "csm_i": np.ascontiguousarray(csm_i, np.float32),
            "kx": np.ascontiguousarray(traj[sl, 0, t], np.float32),
            "ky": np.ascontiguousarray(traj[sl, 1, t], np.float32),
            "cxt": np.ascontiguousarray(cxt, np.float32),
            "cyt": np.ascontiguousarray(cyt, np.float32),
            "cxw": np.ascontiguousarray(cxw, np.float32),
            "cyw": np.ascontiguousarray(cyw, np.float32),
        })
    return in_maps


def combine_outputs(results):
    out = np.zeros((NCOIL, NS), np.complex64)
    for core, res in enumerate(results):
        t, h = divmod(core, 2)
        sl = slice(h * S, (h + 1) * S)
        out[:, sl] += res["out_r"].astype(np.complex64) + 1j * res["out_i"].astype(
            np.complex64)
    return out


def kernel(**inputs) -> np.ndarray:
    from concourse.bass_utils import run_bass_kernel_spmd

    nc = _get_nc()
    in_maps = make_in_maps(**inputs)
    res = run_bass_kernel_spmd(nc, in_maps, core_ids=list(range(8)))
    return combine_outputs(res.results)



# revision 10
# speedup vs baseline: 1.5198x; 1.5198x over previous
"""Batchelor GPU-NUFFT forward operator on 8 Trainium2 NeuronCores.

Math (per timepoint t):
    warped  = bilinear_warp(image, flow[..., t])
    coil    = csm * warped                                  [Nc,Nx,Ny]
    out_t[c,s] = sum_{x,y} coil[c,x,y] exp(-2pi i (kx_s (x-64) + ky_s (y-64)))
    out     = sum_t out_t                                   [Nc,NS] complex64

Sharding: 8 cores = 4 timepoints x 2 sample-halves (4096 samples each).
Host unshard: sum the 4 timepoint partials per half, concat halves.

Device pipeline (per core):
  * warp: host provides the bf16 corner table (DRAM, row (y0*128+x0) holds the
    4 bilinear corners of real+imag), int16 gather indices in the SWDGE
    wrapped layout, and the 4 bilinear weight planes. 16 dma_gather ops land
    the corners directly in [x, y] layout (slot i = y*128 + x); the combine
    and the coil pack run per 4-gather quarter in the gather shadow.
  * NUFFT: Khatri-Rao split y = yo*8 + yi. Per 512-sample chunk, 32
    accumulating bf16 matmuls build PSUM partials Pr = Re(sum coil e^{-iA}),
    Pi = Im(...) directly (stationary blocks Cr | Ci | -Ci make the +- signs
    accumulate in PSUM). The outer phase e^{-iB} is 4 elementwise products,
    folded to 8 coils by +-selector matmuls.
  * trig: phases are range-reduced with custom DVE ops (PHASE_WRAP fuses the
    a0 = kx*(x-64) - 64*ky wrap to one op; ADD_TT_WRAP fuses each chain step
    m2_yi = wrap(m2_parent + ky2^j) with a log-depth parent tree; ABS_SUB
    preps cos args as |m|-1/4 since the ACT Sin spline is only valid on
    [-pi, pi]). ky2/ky4 = wrap(2ky), wrap(4ky) come from the host. All four
    1024-wide trig batches are emitted ahead of the MM loop so the Scalar
    engine streams Sin evaluations while the gather runs.
"""

import sys

if "/opt/trn_rl_repo" not in sys.path:
    sys.path.insert(0, "/opt/trn_rl_repo")

import math

import numpy as np
import ml_dtypes

import concourse.bass as bass
import concourse.tile as tile
from concourse import bacc
from concourse import mybir
from concourse import dve_ops
from concourse.dve_spec import Spec, Src0, Src1, C0, C1, C2, Zero, maxx

P = 128
NX = 128
NCOIL = 8
NS = 8192
NT = 4
S = 4096           # samples per core (half of NS)
CH = 512           # samples per MM chunk (PSUM bank width)
NCHUNK = S // CH   # 8
BW = 1024          # trig batch width (2 chunks)
NBATCH = S // BW   # 4
YI = 8
YO = 16
NPIX = NX * NX
NGATH = 16
GIDX = NPIX // NGATH   # 1024 indices per gather
ELEM = 128             # bf16 elements per table row = 256 bytes
NABS_SC = 2            # yi < NABS_SC: cos-prep via scalar Abs; else DVE ABS_SUB
GP_OUTER = 0           # gpsimd cannot read PSUM: outer products stay on DVE

F32 = mybir.dt.float32
BF16 = mybir.dt.bfloat16
I16 = mybir.dt.int16
TWO_PI = float(2.0 * math.pi)
MAGIC = 12582912.0  # 1.5*2^23: (x + M) - M == round-to-nearest(x) for f32
ALU = mybir.AluOpType
ACTF = mybir.ActivationFunctionType


# ---------------- custom DVE ops ----------------
def _register_dve_op(name, spec):
    if name in dve_ops._SUB_OPCODE_FOR_NAME:
        for op in dve_ops.OPS:
            if op.name == name:
                return op
        raise RuntimeError(name)
    shas = {}
    for ver in ("v3", "v4"):
        uops = dve_ops.lower(spec, ver=ver)
        shas[ver] = dve_ops.DveOpSpec(
            name=name, opcode=0, uops=uops, rd1_en=dve_ops.has_src1(spec)
        ).sha(ver)
    op = dve_ops.DveOp(name, spec, subdim=False, uops_sha=shas)
    dve_ops.OPS.append(op)
    dve_ops._SUB_OPCODE_FOR_NAME[name] = (
        dve_ops._CUSTOM_DVE_ROW_BASE + len(dve_ops.OPS) - 1
    )
    dve_ops.CUSTOM_DVE_SPECS[name] = spec
    return op


def _wrap_np(v):
    return (v - np.round(v)).astype(np.float32)


# out = m - round(m), m = in0*s0 + in1*s1  (s0 may be a [P,1] AP)
_pw_m = Src0 * C0 + Src1 * C1
_pw_r = (_pw_m + C2) - C2
PHASE_WRAP = _register_dve_op(
    "PHASE_WRAP_ANT",
    Spec(
        body=_pw_m - _pw_r,
        reference=lambda in0, in1, s0, s1, imm2: (
            (in0 * s0 + in1 * s1)
            - (((in0 * s0 + in1 * s1) + imm2) - imm2)
        ).astype(np.float32),
    ),
)

# out = y - ((y > .5) - (y < -.5)), y = in0 + in1 : one-period wrap of a sum
_aw_y = Src0 + Src1
ADD_TT_WRAP = _register_dve_op(
    "ADD_TT_WRAP_ANT",
    Spec(
        body=_aw_y + C2 * ((_aw_y < (Zero - C1)) - (C1 < _aw_y)),
        reference=lambda in0, in1, s0, s1, imm2: (
            (in0 + in1)
            + imm2
            * (
                ((in0 + in1) < -s1).astype(np.float32)
                - ((in0 + in1) > s1).astype(np.float32)
            )
        ).astype(np.float32),
    ),
)

# out = |in0| + s0
ABS_SUB = _register_dve_op(
    "ABS_ADD_ANT",
    Spec(
        body=maxx(Src0, Zero - Src0) + C0,
        reference=lambda in0, in1, s0, s1, imm2: (np.abs(in0) + s0).astype(
            np.float32
        ),
    ),
)


def build_program(nc: bass.Bass, dbg: bool = False):
    def dbg_out(name, src_ap, shape, dtype=F32):
        if not dbg:
            return
        d = nc.dram_tensor("dbg_" + name, shape, dtype, kind="ExternalOutput").ap()
        nc.sync.dma_start(d[:], src_ap)

    csm_r = nc.dram_tensor("csm_r", [NCOIL, NX, NX], F32, kind="ExternalInput").ap()
    csm_i = nc.dram_tensor("csm_i", [NCOIL, NX, NX], F32, kind="ExternalInput").ap()
    kx_d = nc.dram_tensor("kx", [S], F32, kind="ExternalInput").ap()
    ky_d = nc.dram_tensor("ky", [S], F32, kind="ExternalInput").ap()
    tbl_d = nc.dram_tensor("tbl", [NPIX, ELEM], BF16, kind="ExternalInput").ap()
    idx_d = nc.dram_tensor("idx", [P, GIDX], I16, kind="ExternalInput").ap()
    w4_d = nc.dram_tensor("w4", [P, NX, 4], F32, kind="ExternalInput").ap()
    out_r = nc.dram_tensor("out_r", [NCOIL, S], F32, kind="ExternalOutput").ap()
    out_i = nc.dram_tensor("out_i", [NCOIL, S], F32, kind="ExternalOutput").ap()

    # ---------------- inline constants ----------------
    pvals = np.arange(P, dtype=np.float32)
    xc_d = nc.inline_tensor((pvals - 64.0).reshape(P, 1), name="c_xc").ap()
    yo8_d = nc.inline_tensor((8.0 * (np.arange(P) % 16)).astype(np.float32)
                             .reshape(P, 1), name="c_yo8").ap()
    half_pi_d = nc.inline_tensor(np.full((P, 1), math.pi / 2, np.float32),
                                 name="c_half_pi").ap()
    sel_np = (np.arange(P)[:, None] // 16 == np.arange(NCOIL)[None, :]).astype(
        np.float32)
    selpm_np = np.concatenate([sel_np, -sel_np], axis=1)  # [128, 16]: +sel | -sel
    selpm_d = nc.inline_tensor(selpm_np, name="c_selpm").ap()

    with tile.TileContext(nc) as tc, \
         tc.tile_pool(name="pp", bufs=1) as pp:

        # --- persistent constants / inputs ---
        idx16 = pp.tile([P, GIDX], I16)
        nc.sync.dma_start(idx16[:], idx_d[:])
        xc_col = pp.tile([P, 1], F32)
        nc.sync.dma_start(xc_col[:], xc_d[:])
        yo8 = pp.tile([P, 1], F32)
        nc.sync.dma_start(yo8[:], yo8_d[:])
        half_pi = pp.tile([P, 1], F32)
        nc.sync.dma_start(half_pi[:], half_pi_d[:])
        selpm32 = pp.tile([P, 2 * NCOIL], F32)
        nc.sync.dma_start(selpm32[:], selpm_d[:])
        selpm = pp.tile([P, 2 * NCOIL], BF16)
        nc.vector.tensor_copy(selpm[:], selpm32[:])

        # packed coil stationary: blocks [Cr | Ci | -Ci], col = c*16 + yo
        RA = pp.tile([P, YI, 3 * P], BF16)

        # --- pools (gp innermost so it can close after the warp) ---
        lp_ctx = tc.tile_pool(name="loop", bufs=1)
        lp = lp_ctx.__enter__()
        kp_ctx = tc.tile_pool(name="kr", bufs=1)
        kp = kp_ctx.__enter__()
        gp_pool_ctx = tc.tile_pool(name="gp", bufs=1)
        gp = gp_pool_ctx.__enter__()
        g8p = gp.tile([P, NX, ELEM], BF16)
        gsems = [nc.alloc_semaphore(f"gath_sem{q}") for q in range(4)]
        for h in range(NGATH):
            nc.gpsimd.dma_gather(
                out_ap=g8p[:, h * 8:(h + 1) * 8, :],
                in_ap=tbl_d[:],
                idxs_ap=idx16[:, h * 64:(h + 1) * 64],
                num_idxs=GIDX,
                num_idxs_reg=GIDX,
                elem_size=ELEM,
                queue_num=h % 4,
            ).then_inc(gsems[h % 4], 16)

        w4sb = gp.tile([P, NX, 4], F32)
        nc.sync.dma_start(w4sb[:], w4_d[:])
        csm_r_sb = gp.tile([P, NCOIL, NX], F32)
        nc.sync.dma_start(csm_r_sb[:], csm_r.rearrange("c x y -> x c y"))
        csm_i_sb = gp.tile([P, NCOIL, NX], F32)
        nc.sync.dma_start(csm_i_sb[:], csm_i.rearrange("c x y -> x c y"))

        # ---------------- trig batches ----------------
        trig = {}

        def emit_trig(b):
            cs = slice(b * BW, (b + 1) * BW)

            def bcast(name, dram):
                t = lp.tile([P, BW], F32, tag=name, bufs=2)
                nc.sync.dma_start(
                    t[:], dram[cs].rearrange("(p s) -> p s", p=1)
                    .to_broadcast([P, BW]))
                return t

            kxc = bcast("kxc", kx_d)
            kyc = bcast("kyc", ky_d)

            # outer phase: m2o = wrap(ky*yo8); aic = -sin(2pi m2o), arc = cos
            m2o = lp.tile([P, BW], F32, tag="m2o", bufs=1)
            nc.vector._custom_dve(PHASE_WRAP, out=m2o[:], in0=kyc[:], in1=kyc[:],
                                  s0=yo8[:, 0:1], s1=0.0, imm2=MAGIC)
            mok = lp.tile([P, BW], F32, tag="mok", bufs=1)
            nc.vector._custom_dve(ABS_SUB, out=mok[:], in0=m2o[:], s0=-0.25)
            aic = kp.tile([P, BW], BF16, tag="aic", bufs=2)
            nc.scalar.activation(aic[:], m2o[:], ACTF.Sin, scale=-TWO_PI)
            arc = kp.tile([P, BW], BF16, tag="arc", bufs=2)
            nc.scalar.activation(arc[:], mok[:], ACTF.Sin, scale=-TWO_PI)

            # inner phase chain, log-depth parent tree
            m2a = lp.tile([P, BW], F32, tag="m2a", bufs=2)
            nc.vector._custom_dve(PHASE_WRAP, out=m2a[:], in0=kxc[:], in1=kyc[:],
                                  s0=xc_col[:, 0:1], s1=-64.0, imm2=MAGIC)
            m2 = {0: m2a}
            kits, krts = [], []
            for yi in range(YI):
                if yi > 0:
                    t = lp.tile([P, BW], F32, tag="m2c", bufs=3)
                    nc.vector._custom_dve(ADD_TT_WRAP, out=t[:],
                                          in0=m2[yi - 1][:],
                                          in1=kyc[:], s1=0.5, imm2=1.0)
                    m2[yi] = t
                kit = kp.tile([P, BW], BF16, tag=f"kit{yi}", bufs=2)
                nc.scalar.activation(kit[:], m2[yi][:], ACTF.Sin, scale=-TWO_PI)
                krt = kp.tile([P, BW], BF16, tag=f"krt{yi}", bufs=2)
                if yi < NABS_SC:
                    mabs = lp.tile([P, BW], F32, tag="mabs", bufs=1)
                    nc.scalar.activation(mabs[:], m2[yi][:], ACTF.Abs)
                    nc.scalar.activation(krt[:], mabs[:], ACTF.Sin,
                                         scale=-TWO_PI, bias=half_pi[:, 0:1])
                else:
                    mk = lp.tile([P, BW], F32, tag="mk", bufs=2)
                    nc.vector._custom_dve(ABS_SUB, out=mk[:], in0=m2[yi][:],
                                          s0=-0.25)
                    nc.scalar.activation(krt[:], mk[:], ACTF.Sin, scale=-TWO_PI)
                kits.append(kit)
                krts.append(krt)
            trig[b] = (kits, krts, arc, aic)
            if b == 0:
                dbg_out("m2a", m2a[:], [P, BW])
                dbg_out("kit0", kits[0][:], [P, BW], BF16)
                dbg_out("krt0", krts[0][:], [P, BW], BF16)
                dbg_out("kit7", kits[7][:], [P, BW], BF16)
                dbg_out("krt7", krts[7][:], [P, BW], BF16)
                dbg_out("aic", aic[:], [P, BW], BF16)
                dbg_out("arc", arc[:], [P, BW], BF16)

        # ---------------- warp quarter: combine + pack ----------------
        def emit_quarter(q):
            ys = slice(32 * q, 32 * q + 32)
            for j in range(4):
                nc.vector.wait_ge(gsems[j], 16 * (q + 1))
            t8r = gp.tile([P, 32, 4], F32, tag="t8r", bufs=2)
            nc.vector.tensor_tensor(t8r[:], g8p[:, ys, 0:4], w4sb[:, ys, :],
                                    op=ALU.mult)
            warped_r = gp.tile([P, 32], F32, tag="wr", bufs=2)
            nc.vector.reduce_sum(warped_r[:], t8r[:], axis=mybir.AxisListType.X)
            t8i = gp.tile([P, 32, 4], F32, tag="t8i", bufs=2)
            nc.vector.tensor_tensor(t8i[:], g8p[:, ys, 4:8], w4sb[:, ys, :],
                                    op=ALU.mult)
            warped_i = gp.tile([P, 32], F32, tag="wi", bufs=2)
            nc.vector.reduce_sum(warped_i[:], t8i[:], axis=mybir.AxisListType.X)

            wr_b = warped_r[:].rearrange("p (c y) -> p c y", c=1).to_broadcast(
                [P, NCOIL, 32])
            wi_b = warped_i[:].rearrange("p (c y) -> p c y", c=1).to_broadcast(
                [P, NCOIL, 32])
            csr = csm_r_sb[:, :, ys]
            csi = csm_i_sb[:, :, ys]

            # RA views for this quarter: [p, c, yo(4), yi(8)]
            ra5 = RA[:].rearrange("p yi (b c yo) -> p b c yo yi", b=3, c=NCOIL)
            cr_v = ra5[:, 0, :, 4 * q:4 * q + 4, :]
            ci_v = ra5[:, 1, :, 4 * q:4 * q + 4, :]
            cin_v = ra5[:, 2, :, 4 * q:4 * q + 4, :]

            def as4(t):
                return t.rearrange("p c (yo yi) -> p c yo yi", yi=YI)

            tt1 = gp.tile([P, NCOIL, 32], F32, tag="tt1", bufs=2)
            nc.vector.tensor_tensor(tt1[:], csr, wr_b, op=ALU.mult)
            tt2 = gp.tile([P, NCOIL, 32], F32, tag="tt2", bufs=2)
            nc.vector.tensor_tensor(tt2[:], csi, wi_b, op=ALU.mult)
            nc.vector.tensor_tensor(cr_v, as4(tt1[:]), as4(tt2[:]),
                                    op=ALU.subtract)
            tt3 = gp.tile([P, NCOIL, 32], F32, tag="tt1", bufs=2)
            nc.vector.tensor_tensor(tt3[:], csr, wi_b, op=ALU.mult)
            tt4 = gp.tile([P, NCOIL, 32], F32, tag="tt2", bufs=2)
            nc.vector.tensor_tensor(tt4[:], csi, wr_b, op=ALU.mult)
            cit = gp.tile([P, NCOIL, 32], F32, tag="cit", bufs=2)
            nc.vector.tensor_tensor(cit[:], tt3[:], tt4[:], op=ALU.add)
            nc.vector.tensor_copy(ci_v, as4(cit[:]))
            nc.vector.tensor_scalar(cin_v, as4(cit[:]), -1.0, None, op0=ALU.mult)

        # ---------------- MM chunk ----------------
        ps_ctx = tc.tile_pool(name="ps", bufs=1, space="PSUM")
        ps = ps_ctx.__enter__()
        pso_ctx = tc.tile_pool(name="pso", bufs=1, space="PSUM")
        pso = pso_ctx.__enter__()

        live = {}

        def emit_mains(ch):
            b, half = divmod(ch, 2)
            sl = slice(half * CH, (half + 1) * CH)
            kits, krts, arc, aic = trig[b]
            Pr = ps.tile([P, CH], F32, tag="Pr", bufs=2)
            Pi = ps.tile([P, CH], F32, tag="Pi", bufs=2)
            for yi in range(YI):
                st, sp = (yi == 0), (yi == YI - 1)
                krt_s = krts[yi][:, sl]
                kit_s = kits[yi][:, sl]
                nc.tensor.matmul(Pr[:], RA[:, yi, 0:128], krt_s,
                                 start=st, stop=False)
                nc.tensor.matmul(Pi[:], RA[:, yi, 0:128], kit_s,
                                 start=st, stop=False)
                nc.tensor.matmul(Pr[:], RA[:, yi, 256:384], kit_s,
                                 start=False, stop=sp)
                nc.tensor.matmul(Pi[:], RA[:, yi, 128:256], krt_s,
                                 start=False, stop=sp)
            live[ch] = (Pr, Pi, arc, aic, sl)

        def emit_post(ch):
            c0 = ch * CH
            Pr, Pi, arc, aic, sl = live.pop(ch)
            q1 = lp.tile([P, CH], BF16, tag="q1", bufs=2)
            nc.vector.tensor_tensor(q1[:], Pr[:], arc[:, sl], op=ALU.mult)
            q2 = lp.tile([P, CH], BF16, tag="q2", bufs=2)
            nc.vector.tensor_tensor(q2[:], Pi[:], aic[:, sl], op=ALU.mult)
            eng3 = nc.gpsimd if GP_OUTER >= 1 else nc.vector
            eng4 = nc.gpsimd if GP_OUTER >= 2 else nc.vector
            q3 = lp.tile([P, CH], BF16, tag="q3", bufs=2)
            eng3.tensor_tensor(q3[:], Pi[:], arc[:, sl], op=ALU.mult)
            q4 = lp.tile([P, CH], BF16, tag="q4", bufs=2)
            eng4.tensor_tensor(q4[:], Pr[:], aic[:, sl], op=ALU.mult)

            SP, SM = selpm[:, 0:NCOIL], selpm[:, NCOIL:2 * NCOIL]
            por = pso.tile([NCOIL, CH], F32, tag="por", bufs=2)
            nc.tensor.matmul(por[:], SP, q1[:], start=True, stop=False)
            nc.tensor.matmul(por[:], SM, q2[:], start=False, stop=True)
            poi = pso.tile([NCOIL, CH], F32, tag="poi", bufs=2)
            nc.tensor.matmul(poi[:], SP, q3[:], start=True, stop=False)
            nc.tensor.matmul(poi[:], SP, q4[:], start=False, stop=True)
            osr = lp.tile([NCOIL, CH], F32, tag="osr", bufs=2)
            nc.scalar.copy(osr[:], por[:])
            osi = lp.tile([NCOIL, CH], F32, tag="osi", bufs=2)
            nc.scalar.copy(osi[:], poi[:])
            nc.sync.dma_start(out_r[:, c0:c0 + CH], osr[:])
            nc.sync.dma_start(out_i[:, c0:c0 + CH], osi[:])

        # ---------------- emission schedule ----------------
        emit_trig(0)
        emit_trig(1)
        emit_quarter(0)
        emit_quarter(1)
        emit_trig(2)
        emit_quarter(2)
        emit_quarter(3)
        dbg_out("RA", RA[:].rearrange("p yi c -> p (yi c)"), [P, YI * 3 * P],
                BF16)
        gp_pool_ctx.__exit__(None, None, None)
        emit_trig(3)

        for ch in range(NCHUNK):
            emit_mains(ch)
            if ch > 0:
                emit_post(ch - 1)
        emit_post(NCHUNK - 1)

        pso_ctx.__exit__(None, None, None)
        ps_ctx.__exit__(None, None, None)
        kp_ctx.__exit__(None, None, None)
        lp_ctx.__exit__(None, None, None)


_COMPILED = {}


def _get_nc(dbg: bool = False):
    key = ("nc", dbg)
    if key not in _COMPILED:
        nc = bacc.Bacc("TRN2", debug=False, num_swdge_queues=4)
        build_program(nc, dbg=dbg)
        nc.compile()
        _COMPILED[key] = nc
    return _COMPILED[key]


# slot g = 16*j + (p%16)  <->  output pixel (x = g%128, y = g//128);
# gather h covers slots [1024h, 1024(h+1)) -> partitions x, columns y.
_Jg = np.arange(GIDX)[None, :]
_Pg = np.arange(P)[:, None]
_G = 16 * _Jg + (_Pg % 16)            # [128, 1024]
_XG = (_G % 128).astype(np.int64)
_YG = (_G // 128).astype(np.int64)
_BF16 = ml_dtypes.bfloat16


def _build_tables(image_r, image_i, flow):
    """Per-timepoint: corner table (bf16, row y0*128+x0), idx16, weights."""
    ir = np.ascontiguousarray(image_r, np.float32)
    ii = np.ascontiguousarray(image_i, np.float32)
    irT, iiT = ir.T, ii.T                     # [y, x]
    y1 = np.minimum(np.arange(NX) + 1, NX - 1)
    x1 = np.minimum(np.arange(NX) + 1, NX - 1)
    tables = []
    for t in range(NT):
        f0 = np.asarray(flow[:, :, 0, t], np.float32)
        f1 = np.asarray(flow[:, :, 1, t], np.float32)
        # float32 math mirrors the jax reference exactly
        xg = np.arange(NX, dtype=np.float32)[:, None]
        yg = np.arange(NX, dtype=np.float32)[None, :]
        cx = np.clip(xg + f0, np.float32(0.0), np.float32(NX - 1))
        cy = np.clip(yg + f1, np.float32(0.0), np.float32(NX - 1))
        x0 = np.floor(cx)
        y0 = np.floor(cy)
        wx = (cx - x0).astype(np.float32)     # [x, y]
        wy = (cy - y0).astype(np.float32)
        w4 = np.stack([(1 - wx) * (1 - wy), (1 - wx) * wy,
                       wx * (1 - wy), wx * wy], axis=-1).astype(np.float32)
        x0i = x0.astype(np.int64)
        y0i = y0.astype(np.int64)
        idxv = (y0i * NX + x0i).astype(np.int16)      # [x, y]
        idx16 = idxv[_XG, _YG]                        # wrapped gather layout

        tbl = np.zeros((NX, NX, ELEM), dtype=_BF16)
        tbl[:, :, 0] = irT
        tbl[:, :, 1] = irT[y1, :]
        tbl[:, :, 2] = irT[:, x1]
        tbl[:, :, 3] = irT[y1][:, x1]
        tbl[:, :, 4] = iiT
        tbl[:, :, 5] = iiT[y1, :]
        tbl[:, :, 6] = iiT[:, x1]
        tbl[:, :, 7] = iiT[y1][:, x1]
        tables.append((tbl.reshape(NPIX, ELEM), idx16, w4))
    return tables


def make_in_maps(image_r, image_i, csm_r, csm_i, traj, dcf, flow):
    del dcf  # unused by the operator
    tables = _build_tables(image_r, image_i, flow)
    csm_r = np.ascontiguousarray(csm_r, np.float32)
    csm_i = np.ascontiguousarray(csm_i, np.float32)
    in_maps = []
    for core in range(8):
        t, h = divmod(core, 2)
        sl = slice(h * S, (h + 1) * S)
        tbl, idx16, w4 = tables[t]
        in_maps.append({
            "csm_r": csm_r,
            "csm_i": csm_i,
            "kx": np.ascontiguousarray(traj[sl, 0, t], np.float32),
            "ky": np.ascontiguousarray(traj[sl, 1, t], np.float32),
            "tbl": np.ascontiguousarray(tbl),
            "idx": np.ascontiguousarray(idx16),
            "w4": np.ascontiguousarray(w4),
        })
    return in_maps


def combine_outputs(results):
    out = np.zeros((NCOIL, NS), np.complex64)
    for core, res in enumerate(results):
        t, h = divmod(core, 2)
        sl = slice(h * S, (h + 1) * S)
        out[:, sl] += res["out_r"].astype(np.complex64) + 1j * res["out_i"].astype(
            np.complex64)
    return out


def kernel(**inputs) -> np.ndarray:
    from concourse.bass_utils import run_bass_kernel_spmd

    nc = _get_nc()
    in_maps = make_in_maps(**inputs)
    res = run_bass_kernel_spmd(nc, in_maps, core_ids=list(range(8)))
    return combine_outputs(res.results)


# revision 13
# speedup vs baseline: 1.5265x; 1.0044x over previous
"""Batchelor GPU-NUFFT forward operator on 8 Trainium2 NeuronCores.

Math (per timepoint t):
    warped  = bilinear_warp(image, flow[..., t])
    coil    = csm * warped                                  [Nc,Nx,Ny]
    out_t[c,s] = sum_{x,y} coil[c,x,y] exp(-2pi i (kx_s (x-64) + ky_s (y-64)))
    out     = sum_t out_t                                   [Nc,NS] complex64

Sharding: 8 cores = 4 timepoints x 2 sample-halves (4096 samples each).
Host unshard: sum the 4 timepoint partials per half, concat halves.

Device pipeline (per core):
  * warp: host provides the bf16 corner table (DRAM, row (y0*128+x0) holds the
    4 bilinear corners of real+imag), int16 gather indices in the SWDGE
    wrapped layout, and the 4 bilinear weight planes. 16 dma_gather ops land
    the corners directly in [x, y] layout (slot i = y*128 + x); the combine
    and the coil pack run per 4-gather quarter in the gather shadow.
  * NUFFT: Khatri-Rao split y = yo*8 + yi. Per 512-sample chunk, 32
    accumulating bf16 matmuls build PSUM partials Pr = Re(sum coil e^{-iA}),
    Pi = Im(...) directly (stationary blocks Cr | Ci | -Ci make the +- signs
    accumulate in PSUM). The outer phase e^{-iB} is 4 elementwise products,
    folded to 8 coils by +-selector matmuls.
  * trig: phases are range-reduced with custom DVE ops (PHASE_WRAP fuses the
    a0 = kx*(x-64) - 64*ky wrap to one op; ADD_TT_WRAP fuses each chain step
    m2_yi = wrap(m2_parent + ky2^j) with a log-depth parent tree; ABS_SUB
    preps cos args as |m|-1/4 since the ACT Sin spline is only valid on
    [-pi, pi]). ky2/ky4 = wrap(2ky), wrap(4ky) come from the host. All four
    1024-wide trig batches are emitted ahead of the MM loop so the Scalar
    engine streams Sin evaluations while the gather runs.
"""

import sys

if "/opt/trn_rl_repo" not in sys.path:
    sys.path.insert(0, "/opt/trn_rl_repo")

import math

import numpy as np
import ml_dtypes

import concourse.bass as bass
import concourse.tile as tile
from concourse import bacc
from concourse import mybir
from concourse import dve_ops
from concourse.dve_spec import Spec, Src0, Src1, C0, C1, C2, Zero, maxx

P = 128
NX = 128
NCOIL = 8
NS = 8192
NT = 4
S = 4096           # samples per core (half of NS)
CH = 512           # samples per MM chunk (PSUM bank width)
NCHUNK = S // CH   # 8
BW = 1024          # trig batch width (2 chunks)
NBATCH = S // BW   # 4
YI = 8
YO = 16
NPIX = NX * NX
NGATH = 16
GIDX = NPIX // NGATH   # 1024 indices per gather
ELEM = 128             # bf16 elements per table row = 256 bytes
NABS_SC = 2            # yi < NABS_SC: cos-prep via scalar Abs; else DVE ABS_SUB
GP_OUTER = 0           # gpsimd cannot read PSUM: outer products stay on DVE

F32 = mybir.dt.float32
BF16 = mybir.dt.bfloat16
I16 = mybir.dt.int16
TWO_PI = float(2.0 * math.pi)
MAGIC = 12582912.0  # 1.5*2^23: (x + M) - M == round-to-nearest(x) for f32
ALU = mybir.AluOpType
ACTF = mybir.ActivationFunctionType


# ---------------- custom DVE ops ----------------
def _register_dve_op(name, spec):
    if name in dve_ops._SUB_OPCODE_FOR_NAME:
        for op in dve_ops.OPS:
            if op.name == name:
                return op
        raise RuntimeError(name)
    shas = {}
    for ver in ("v3", "v4"):
        uops = dve_ops.lower(spec, ver=ver)
        shas[ver] = dve_ops.DveOpSpec(
            name=name, opcode=0, uops=uops, rd1_en=dve_ops.has_src1(spec)
        ).sha(ver)
    op = dve_ops.DveOp(name, spec, subdim=False, uops_sha=shas)
    dve_ops.OPS.append(op)
    dve_ops._SUB_OPCODE_FOR_NAME[name] = (
        dve_ops._CUSTOM_DVE_ROW_BASE + len(dve_ops.OPS) - 1
    )
    dve_ops.CUSTOM_DVE_SPECS[name] = spec
    return op


def _wrap_np(v):
    return (v - np.round(v)).astype(np.float32)


# out = m - round(m), m = in0*s0 + in1*s1  (s0 may be a [P,1] AP)
_pw_m = Src0 * C0 + Src1 * C1
_pw_r = (_pw_m + C2) - C2
PHASE_WRAP = _register_dve_op(
    "PHASE_WRAP_ANT",
    Spec(
        body=_pw_m - _pw_r,
        reference=lambda in0, in1, s0, s1, imm2: (
            (in0 * s0 + in1 * s1)
            - (((in0 * s0 + in1 * s1) + imm2) - imm2)
        ).astype(np.float32),
    ),
)

# out = y - ((y > .5) - (y < -.5)), y = in0 + in1 : one-period wrap of a sum
_aw_y = Src0 + Src1
ADD_TT_WRAP = _register_dve_op(
    "ADD_TT_WRAP_ANT",
    Spec(
        body=_aw_y + C2 * ((_aw_y < (Zero - C1)) - (C1 < _aw_y)),
        reference=lambda in0, in1, s0, s1, imm2: (
            (in0 + in1)
            + imm2
            * (
                ((in0 + in1) < -s1).astype(np.float32)
                - ((in0 + in1) > s1).astype(np.float32)
            )
        ).astype(np.float32),
    ),
)

# out = |in0| + s0
ABS_SUB = _register_dve_op(
    "ABS_ADD_ANT",
    Spec(
        body=maxx(Src0, Zero - Src0) + C0,
        reference=lambda in0, in1, s0, s1, imm2: (np.abs(in0) + s0).astype(
            np.float32
        ),
    ),
)


def build_program(nc: bass.Bass, dbg: bool = False):
    def dbg_out(name, src_ap, shape, dtype=F32):
        if not dbg:
            return
        d = nc.dram_tensor("dbg_" + name, shape, dtype, kind="ExternalOutput").ap()
        nc.sync.dma_start(d[:], src_ap)

    csm_r = nc.dram_tensor("csm_r", [NCOIL, NX, NX], F32, kind="ExternalInput").ap()
    csm_i = nc.dram_tensor("csm_i", [NCOIL, NX, NX], F32, kind="ExternalInput").ap()
    kx_d = nc.dram_tensor("kx", [S], F32, kind="ExternalInput").ap()
    ky_d = nc.dram_tensor("ky", [S], F32, kind="ExternalInput").ap()
    tbl_d = nc.dram_tensor("tbl", [NPIX, ELEM], BF16, kind="ExternalInput").ap()
    idx_d = nc.dram_tensor("idx", [P, GIDX], I16, kind="ExternalInput").ap()
    w4_d = nc.dram_tensor("w4", [P, NX, 4], F32, kind="ExternalInput").ap()
    out_r = nc.dram_tensor("out_r", [NCOIL, S], F32, kind="ExternalOutput").ap()
    out_i = nc.dram_tensor("out_i", [NCOIL, S], F32, kind="ExternalOutput").ap()

    # ---------------- inline constants ----------------
    pvals = np.arange(P, dtype=np.float32)
    xc_d = nc.inline_tensor((pvals - 64.0).reshape(P, 1), name="c_xc").ap()
    yo8_d = nc.inline_tensor((8.0 * (np.arange(P) % 16)).astype(np.float32)
                             .reshape(P, 1), name="c_yo8").ap()
    half_pi_d = nc.inline_tensor(np.full((P, 1), math.pi / 2, np.float32),
                                 name="c_half_pi").ap()
    sel_np = (np.arange(P)[:, None] // 16 == np.arange(NCOIL)[None, :]).astype(
        np.float32)
    selpm_np = np.concatenate([sel_np, -sel_np], axis=1)  # [128, 16]: +sel | -sel
    selpm_d = nc.inline_tensor(selpm_np, name="c_selpm").ap()

    with tile.TileContext(nc) as tc, \
         tc.tile_pool(name="pp", bufs=1) as pp:

        # --- persistent constants / inputs ---
        idx16 = pp.tile([P, GIDX], I16)
        nc.sync.dma_start(idx16[:], idx_d[:])
        kxb = pp.tile([P, S], F32)
        nc.sync.dma_start(
            kxb[:], kx_d[:].rearrange("(p s) -> p s", p=1).to_broadcast([P, S]))
        kyb = pp.tile([P, S], F32)
        nc.sync.dma_start(
            kyb[:], ky_d[:].rearrange("(p s) -> p s", p=1).to_broadcast([P, S]))
        xc_col = pp.tile([P, 1], F32)
        nc.sync.dma_start(xc_col[:], xc_d[:])
        yo8 = pp.tile([P, 1], F32)
        nc.sync.dma_start(yo8[:], yo8_d[:])
        half_pi = pp.tile([P, 1], F32)
        nc.sync.dma_start(half_pi[:], half_pi_d[:])
        selpm32 = pp.tile([P, 2 * NCOIL], F32)
        nc.sync.dma_start(selpm32[:], selpm_d[:])
        selpm = pp.tile([P, 2 * NCOIL], BF16)
        nc.vector.tensor_copy(selpm[:], selpm32[:])

        # packed coil stationary: blocks [Cr | Ci | -Ci], col = c*16 + yo
        RA = pp.tile([P, YI, 3 * P], BF16)

        # --- pools (gp innermost so it can close after the warp) ---
        lp_ctx = tc.tile_pool(name="loop", bufs=1)
        lp = lp_ctx.__enter__()
        kp_ctx = tc.tile_pool(name="kr", bufs=1)
        kp = kp_ctx.__enter__()
        gp_pool_ctx = tc.tile_pool(name="gp", bufs=1)
        gp = gp_pool_ctx.__enter__()
        g8p = gp.tile([P, NX, ELEM], BF16)
        gsems = [nc.alloc_semaphore(f"gath_sem{q}") for q in range(4)]
        for h in range(NGATH):
            nc.gpsimd.dma_gather(
                out_ap=g8p[:, h * 8:(h + 1) * 8, :],
                in_ap=tbl_d[:],
                idxs_ap=idx16[:, h * 64:(h + 1) * 64],
                num_idxs=GIDX,
                num_idxs_reg=GIDX,
                elem_size=ELEM,
                queue_num=h % 4,
            ).then_inc(gsems[h % 4], 16)

        w4sb = gp.tile([P, NX, 4], F32)
        nc.sync.dma_start(w4sb[:], w4_d[:])
        csm_r_sb = gp.tile([P, NCOIL, NX], F32)
        nc.sync.dma_start(csm_r_sb[:], csm_r.rearrange("c x y -> x c y"))
        csm_i_sb = gp.tile([P, NCOIL, NX], F32)
        nc.sync.dma_start(csm_i_sb[:], csm_i.rearrange("c x y -> x c y"))

        # ---------------- trig batches (emitted in pieces) ----------------
        trig = {}

        def make_trig(b):
            cs = slice(b * BW, (b + 1) * BW)
            st = {"m2": {}, "kits": [], "krts": []}

            def emit_yi(yi):
                m2 = st["m2"]
                kyc = st["kyc"]
                if yi > 0:
                    t = lp.tile([P, BW], F32, tag="m2c", bufs=2)
                    nc.vector._custom_dve(ADD_TT_WRAP, out=t[:],
                                          in0=m2[yi - 1][:],
                                          in1=kyc, s1=0.5, imm2=1.0)
                    m2[yi] = t
                kit = kp.tile([P, BW], BF16, tag=f"kit{yi}", bufs=2)
                nc.scalar.activation(kit[:], m2[yi][:], ACTF.Sin, scale=-TWO_PI)
                krt = kp.tile([P, BW], BF16, tag=f"krt{yi}", bufs=2)
                if yi < NABS_SC:
                    mabs = lp.tile([P, BW], F32, tag="mabs", bufs=1)
                    nc.scalar.activation(mabs[:], m2[yi][:], ACTF.Abs)
                    nc.scalar.activation(krt[:], mabs[:], ACTF.Sin,
                                         scale=-TWO_PI, bias=half_pi[:, 0:1])
                else:
                    mk = lp.tile([P, BW], F32, tag="mk", bufs=2)
                    nc.vector._custom_dve(ABS_SUB, out=mk[:], in0=m2[yi][:],
                                          s0=-0.25)
                    nc.scalar.activation(krt[:], mk[:], ACTF.Sin, scale=-TWO_PI)
                st["kits"].append(kit)
                st["krts"].append(krt)

            def piece0():
                kxc = kxb[:, cs]
                kyc = st["kyc"] = kyb[:, cs]
                m2o = lp.tile([P, BW], F32, tag="m2o", bufs=1)
                nc.vector._custom_dve(PHASE_WRAP, out=m2o[:], in0=kyc,
                                      in1=kyc, s0=yo8[:, 0:1], s1=0.0,
                                      imm2=MAGIC)
                mok = lp.tile([P, BW], F32, tag="mok", bufs=1)
                nc.vector._custom_dve(ABS_SUB, out=mok[:], in0=m2o[:], s0=-0.25)
                aic = kp.tile([P, BW], BF16, tag="aic", bufs=2)
                nc.scalar.activation(aic[:], m2o[:], ACTF.Sin, scale=-TWO_PI)
                arc = kp.tile([P, BW], BF16, tag="arc", bufs=2)
                nc.scalar.activation(arc[:], mok[:], ACTF.Sin, scale=-TWO_PI)
                m2a = lp.tile([P, BW], F32, tag="m2a", bufs=1)
                nc.vector._custom_dve(PHASE_WRAP, out=m2a[:], in0=kxc,
                                      in1=kyc, s0=xc_col[:, 0:1], s1=-64.0,
                                      imm2=MAGIC)
                st["m2"][0] = m2a
                emit_yi(0)
                trig[b] = (st["kits"], st["krts"], arc, aic)

            return [piece0] + [lambda yi=yi: emit_yi(yi) for yi in range(1, YI)]

        # ---------------- warp quarter: combine + pack ----------------
        def emit_quarter(q):
            ys = slice(32 * q, 32 * q + 32)
            for j in range(4):
                nc.vector.wait_ge(gsems[j], 16 * (q + 1))
            t8r = gp.tile([P, 32, 4], F32, tag="t8r", bufs=2)
            nc.vector.tensor_tensor(t8r[:], g8p[:, ys, 0:4], w4sb[:, ys, :],
                                    op=ALU.mult)
            warped_r = gp.tile([P, 32], F32, tag="wr", bufs=2)
            nc.vector.reduce_sum(warped_r[:], t8r[:], axis=mybir.AxisListType.X)
            t8i = gp.tile([P, 32, 4], F32, tag="t8i", bufs=2)
            nc.vector.tensor_tensor(t8i[:], g8p[:, ys, 4:8], w4sb[:, ys, :],
                                    op=ALU.mult)
            warped_i = gp.tile([P, 32], F32, tag="wi", bufs=2)
            nc.vector.reduce_sum(warped_i[:], t8i[:], axis=mybir.AxisListType.X)

            wr_b = warped_r[:].rearrange("p (c y) -> p c y", c=1).to_broadcast(
                [P, NCOIL, 32])
            wi_b = warped_i[:].rearrange("p (c y) -> p c y", c=1).to_broadcast(
                [P, NCOIL, 32])
            csr = csm_r_sb[:, :, ys]
            csi = csm_i_sb[:, :, ys]

            # RA views for this quarter: [p, c, yo(4), yi(8)]
            ra5 = RA[:].rearrange("p yi (b c yo) -> p b c yo yi", b=3, c=NCOIL)
            cr_v = ra5[:, 0, :, 4 * q:4 * q + 4, :]
            ci_v = ra5[:, 1, :, 4 * q:4 * q + 4, :]
            cin_v = ra5[:, 2, :, 4 * q:4 * q + 4, :]

            def as4(t):
                return t.rearrange("p c (yo yi) -> p c yo yi", yi=YI)

            tt1 = gp.tile([P, NCOIL, 32], F32, tag="tt1", bufs=2)
            nc.vector.tensor_tensor(tt1[:], csr, wr_b, op=ALU.mult)
            tt2 = gp.tile([P, NCOIL, 32], F32, tag="tt2", bufs=2)
            nc.vector.tensor_tensor(tt2[:], csi, wi_b, op=ALU.mult)
            nc.vector.tensor_tensor(cr_v, as4(tt1[:]), as4(tt2[:]),
                                    op=ALU.subtract)
            tt3 = gp.tile([P, NCOIL, 32], F32, tag="tt1", bufs=2)
            nc.vector.tensor_tensor(tt3[:], csr, wi_b, op=ALU.mult)
            tt4 = gp.tile([P, NCOIL, 32], F32, tag="tt2", bufs=2)
            nc.vector.tensor_tensor(tt4[:], csi, wr_b, op=ALU.mult)
            cit = gp.tile([P, NCOIL, 32], F32, tag="cit", bufs=2)
            nc.vector.tensor_tensor(cit[:], tt3[:], tt4[:], op=ALU.add)
            nc.vector.tensor_copy(ci_v, as4(cit[:]))
            nc.vector.tensor_scalar(cin_v, as4(cit[:]), -1.0, None, op0=ALU.mult)

        # ---------------- MM chunk ----------------
        ps_ctx = tc.tile_pool(name="ps", bufs=1, space="PSUM")
        ps = ps_ctx.__enter__()
        pso_ctx = tc.tile_pool(name="pso", bufs=1, space="PSUM")
        pso = pso_ctx.__enter__()

        live = {}

        def emit_mains(ch):
            b, half = divmod(ch, 2)
            sl = slice(half * CH, (half + 1) * CH)
            kits, krts, arc, aic = trig[b]
            Pr = ps.tile([P, CH], F32, tag="Pr", bufs=2)
            Pi = ps.tile([P, CH], F32, tag="Pi", bufs=2)
            for yi in range(YI):
                st, sp = (yi == 0), (yi == YI - 1)
                krt_s = krts[yi][:, sl]
                kit_s = kits[yi][:, sl]
                nc.tensor.matmul(Pr[:], RA[:, yi, 0:128], krt_s,
                                 start=st, stop=False)
                nc.tensor.matmul(Pi[:], RA[:, yi, 0:128], kit_s,
                                 start=st, stop=False)
                nc.tensor.matmul(Pr[:], RA[:, yi, 256:384], kit_s,
                                 start=False, stop=sp)
                nc.tensor.matmul(Pi[:], RA[:, yi, 128:256], krt_s,
                                 start=False, stop=sp)
            live[ch] = (Pr, Pi, arc, aic, sl)

        def emit_post(ch):
            c0 = ch * CH
            Pr, Pi, arc, aic, sl = live.pop(ch)
            q1 = lp.tile([P, CH], BF16, tag="q1", bufs=2)
            nc.vector.tensor_tensor(q1[:], Pr[:], arc[:, sl], op=ALU.mult)
            q2 = lp.tile([P, CH], BF16, tag="q2", bufs=2)
            nc.vector.tensor_tensor(q2[:], Pi[:], aic[:, sl], op=ALU.mult)
            eng3 = nc.gpsimd if GP_OUTER >= 1 else nc.vector
            eng4 = nc.gpsimd if GP_OUTER >= 2 else nc.vector
            q3 = lp.tile([P, CH], BF16, tag="q3", bufs=2)
            eng3.tensor_tensor(q3[:], Pi[:], arc[:, sl], op=ALU.mult)
            q4 = lp.tile([P, CH], BF16, tag="q4", bufs=2)
            eng4.tensor_tensor(q4[:], Pr[:], aic[:, sl], op=ALU.mult)

            SP, SM = selpm[:, 0:NCOIL], selpm[:, NCOIL:2 * NCOIL]
            por = pso.tile([NCOIL, CH], F32, tag="por", bufs=2)
            nc.tensor.matmul(por[:], SP, q1[:], start=True, stop=False)
            nc.tensor.matmul(por[:], SM, q2[:], start=False, stop=True)
            poi = pso.tile([NCOIL, CH], F32, tag="poi", bufs=2)
            nc.tensor.matmul(poi[:], SP, q3[:], start=True, stop=False)
            nc.tensor.matmul(poi[:], SP, q4[:], start=False, stop=True)
            osr = lp.tile([NCOIL, CH], F32, tag="osr", bufs=1)
            nc.scalar.copy(osr[:], por[:])
            osi = lp.tile([NCOIL, CH], F32, tag="osi", bufs=1)
            nc.scalar.copy(osi[:], poi[:])
            nc.sync.dma_start(out_r[:, c0:c0 + CH], osr[:])
            nc.sync.dma_start(out_i[:, c0:c0 + CH], osi[:])

        # ---------------- emission schedule ----------------
        for p in make_trig(0):
            p()
        for q in range(4):
            emit_quarter(q)
        dbg_out("RA", RA[:].rearrange("p yi c -> p (yi c)"), [P, YI * 3 * P],
                BF16)
        gp_pool_ctx.__exit__(None, None, None)
        for p in make_trig(1):
            p()

        # interleave trig batches 2/3 into the MM loop so the DVE queue never
        # blocks the selector matmuls: b2 pieces land after posts 1-3, b3
        # after posts 3-5.
        t2 = make_trig(2)
        t3 = make_trig(3)
        pieces = {0: t2[0:3], 1: t2[3:6], 2: t2[6:8] + t3[0:1],
                  3: t3[1:4], 4: t3[4:8]}

        for ch in range(NCHUNK):
            emit_mains(ch)
            if ch > 0:
                emit_post(ch - 1)
                for p in pieces.get(ch - 1, []):
                    p()
        emit_post(NCHUNK - 1)

        pso_ctx.__exit__(None, None, None)
        ps_ctx.__exit__(None, None, None)
        kp_ctx.__exit__(None, None, None)
        lp_ctx.__exit__(None, None, None)


_COMPILED = {}


def _get_nc(dbg: bool = False):
    key = ("nc", dbg)
    if key not in _COMPILED:
        nc = bacc.Bacc("TRN2", debug=False, num_swdge_queues=4)
        build_program(nc, dbg=dbg)
        nc.compile()
        _COMPILED[key] = nc
    return _COMPILED[key]


# slot g = 16*j + (p%16)  <->  output pixel (x = g%128, y = g//128);
# gather h covers slots [1024h, 1024(h+1)) -> partitions x, columns y.
_Jg = np.arange(GIDX)[None, :]
_Pg = np.arange(P)[:, None]
_G = 16 * _Jg + (_Pg % 16)            # [128, 1024]
_XG = (_G % 128).astype(np.int64)
_YG = (_G // 128).astype(np.int64)
_BF16 = ml_dtypes.bfloat16


def _build_tables(image_r, image_i, flow):
    """Per-timepoint: corner table (bf16, row y0*128+x0), idx16, weights."""
    ir = np.ascontiguousarray(image_r, np.float32)
    ii = np.ascontiguousarray(image_i, np.float32)
    irT, iiT = ir.T, ii.T                     # [y, x]
    y1 = np.minimum(np.arange(NX) + 1, NX - 1)
    x1 = np.minimum(np.arange(NX) + 1, NX - 1)
    tables = []
    for t in range(NT):
        f0 = np.asarray(flow[:, :, 0, t], np.float32)
        f1 = np.asarray(flow[:, :, 1, t], np.float32)
        # float32 math mirrors the jax reference exactly
        xg = np.arange(NX, dtype=np.float32)[:, None]
        yg = np.arange(NX, dtype=np.float32)[None, :]
        cx = np.clip(xg + f0, np.float32(0.0), np.float32(NX - 1))
        cy = np.clip(yg + f1, np.float32(0.0), np.float32(NX - 1))
        x0 = np.floor(cx)
        y0 = np.floor(cy)
        wx = (cx - x0).astype(np.float32)     # [x, y]
        wy = (cy - y0).astype(np.float32)
        w4 = np.stack([(1 - wx) * (1 - wy), (1 - wx) * wy,
                       wx * (1 - wy), wx * wy], axis=-1).astype(np.float32)
        x0i = x0.astype(np.int64)
        y0i = y0.astype(np.int64)
        idxv = (y0i * NX + x0i).astype(np.int16)      # [x, y]
        idx16 = idxv[_XG, _YG]                        # wrapped gather layout

        tbl = np.zeros((NX, NX, ELEM), dtype=_BF16)
        tbl[:, :, 0] = irT
        tbl[:, :, 1] = irT[y1, :]
        tbl[:, :, 2] = irT[:, x1]
        tbl[:, :, 3] = irT[y1][:, x1]
        tbl[:, :, 4] = iiT
        tbl[:, :, 5] = iiT[y1, :]
        tbl[:, :, 6] = iiT[:, x1]
        tbl[:, :, 7] = iiT[y1][:, x1]
        tables.append((tbl.reshape(NPIX, ELEM), idx16, w4))
    return tables


def make_in_maps(image_r, image_i, csm_r, csm_i, traj, dcf, flow):
    del dcf  # unused by the operator
    tables = _build_tables(image_r, image_i, flow)
    csm_r = np.ascontiguousarray(csm_r, np.float32)
    csm_i = np.ascontiguousarray(csm_i, np.float32)
    in_maps = []
    for core in range(8):
        t, h = divmod(core, 2)
        sl = slice(h * S, (h + 1) * S)
        tbl, idx16, w4 = tables[t]
        in_maps.append({
            "csm_r": csm_r,
            "csm_i": csm_i,
            "kx": np.ascontiguousarray(traj[sl, 0, t], np.float32),
            "ky": np.ascontiguousarray(traj[sl, 1, t], np.float32),
            "tbl": np.ascontiguousarray(tbl),
            "idx": np.ascontiguousarray(idx16),
            "w4": np.ascontiguousarray(w4),
        })
    return in_maps


def combine_outputs(results):
    out = np.zeros((NCOIL, NS), np.complex64)
    for core, res in enumerate(results):
        t, h = divmod(core, 2)
        sl = slice(h * S, (h + 1) * S)
        out[:, sl] += res["out_r"].astype(np.complex64) + 1j * res["out_i"].astype(
            np.complex64)
    return out


def kernel(**inputs) -> np.ndarray:
    from concourse.bass_utils import run_bass_kernel_spmd

    nc = _get_nc()
    in_maps = make_in_maps(**inputs)
    res = run_bass_kernel_spmd(nc, in_maps, core_ids=list(range(8)))
    return combine_outputs(res.results)


# revision 16
# speedup vs baseline: 1.6982x; 1.1124x over previous
"""Batchelor GPU-NUFFT forward operator on 8 Trainium2 NeuronCores.

Math (per timepoint t):
    warped  = bilinear_warp(image, flow[..., t])
    coil    = csm * warped                                  [Nc,Nx,Ny]
    out_t[c,s] = sum_{x,y} coil[c,x,y] exp(-2pi i (kx_s (x-64) + ky_s (y-64)))
    out     = sum_t out_t                                   [Nc,NS] complex64

Sharding: 8 cores = 4 timepoints x 2 sample-halves (4096 samples each).
Host unshard: sum the 4 timepoint partials per half, concat halves.

Device pipeline (per core):
  * warp: host provides the bf16 corner table (DRAM, row (y0*128+x0) holds the
    4 bilinear corners of real+imag), int16 gather indices in the SWDGE
    wrapped layout, and the 4 bilinear weight planes. 16 dma_gather ops land
    the corners directly in [x, y] layout (slot i = y*128 + x); the combine
    and the coil pack run per 4-gather quarter in the gather shadow.
  * NUFFT: Khatri-Rao split y = yo*8 + yi. Per 512-sample chunk, 32
    accumulating bf16 matmuls build PSUM partials Pr = Re(sum coil e^{-iA}),
    Pi = Im(...) directly (stationary blocks Cr | Ci | -Ci make the +- signs
    accumulate in PSUM). The outer phase e^{-iB} is 4 elementwise products,
    folded to 8 coils by +-selector matmuls.
  * trig: phases are range-reduced with custom DVE ops (PHASE_WRAP fuses the
    a0 = kx*(x-64) - 64*ky wrap to one op; ADD_TT_WRAP fuses each chain step
    m2_yi = wrap(m2_parent + ky2^j) with a log-depth parent tree; ABS_SUB
    preps cos args as |m|-1/4 since the ACT Sin spline is only valid on
    [-pi, pi]). ky2/ky4 = wrap(2ky), wrap(4ky) come from the host. All four
    1024-wide trig batches are emitted ahead of the MM loop so the Scalar
    engine streams Sin evaluations while the gather runs.
"""

import sys

if "/opt/trn_rl_repo" not in sys.path:
    sys.path.insert(0, "/opt/trn_rl_repo")

import math

import numpy as np
import ml_dtypes

import concourse.bass as bass
import concourse.tile as tile
from concourse import bacc
from concourse import mybir
from concourse import dve_ops
from concourse.dve_spec import Spec, Src0, Src1, C0, C1, C2, Zero, maxx

P = 128
NX = 128
NCOIL = 8
NS = 8192
NT = 4
S = 4096           # samples per core (half of NS)
CH = 512           # samples per MM chunk (PSUM bank width)
NCHUNK = S // CH   # 8
BW = 1024          # trig batch width (2 chunks)
NBATCH = S // BW   # 4
YI = 8
YO = 16
NPIX = NX * NX
NGATH = 16
GIDX = NPIX // NGATH   # 1024 indices per gather
ELEM = 128             # bf16 elements per table row = 256 bytes
NABS_SC = 2            # yi < NABS_SC: cos-prep via scalar Abs; else DVE ABS_SUB
GP_OUTER = 0           # gpsimd cannot read PSUM: outer products stay on DVE

F32 = mybir.dt.float32
BF16 = mybir.dt.bfloat16
I16 = mybir.dt.int16
TWO_PI = float(2.0 * math.pi)
MAGIC = 12582912.0  # 1.5*2^23: (x + M) - M == round-to-nearest(x) for f32
ALU = mybir.AluOpType
ACTF = mybir.ActivationFunctionType


# ---------------- custom DVE ops ----------------
def _register_dve_op(name, spec):
    if name in dve_ops._SUB_OPCODE_FOR_NAME:
        for op in dve_ops.OPS:
            if op.name == name:
                return op
        raise RuntimeError(name)
    shas = {}
    for ver in ("v3", "v4"):
        uops = dve_ops.lower(spec, ver=ver)
        shas[ver] = dve_ops.DveOpSpec(
            name=name, opcode=0, uops=uops, rd1_en=dve_ops.has_src1(spec)
        ).sha(ver)
    op = dve_ops.DveOp(name, spec, subdim=False, uops_sha=shas)
    dve_ops.OPS.append(op)
    dve_ops._SUB_OPCODE_FOR_NAME[name] = (
        dve_ops._CUSTOM_DVE_ROW_BASE + len(dve_ops.OPS) - 1
    )
    dve_ops.CUSTOM_DVE_SPECS[name] = spec
    return op


def _wrap_np(v):
    return (v - np.round(v)).astype(np.float32)


# out = m - round(m), m = in0*s0 + in1*s1  (s0 may be a [P,1] AP)
_pw_m = Src0 * C0 + Src1 * C1
_pw_r = (_pw_m + C2) - C2
PHASE_WRAP = _register_dve_op(
    "PHASE_WRAP_ANT",
    Spec(
        body=_pw_m - _pw_r,
        reference=lambda in0, in1, s0, s1, imm2: (
            (in0 * s0 + in1 * s1)
            - (((in0 * s0 + in1 * s1) + imm2) - imm2)
        ).astype(np.float32),
    ),
)

# out = y - ((y > .5) - (y < -.5)), y = in0 + in1 : one-period wrap of a sum
_aw_y = Src0 + Src1
ADD_TT_WRAP = _register_dve_op(
    "ADD_TT_WRAP_ANT",
    Spec(
        body=_aw_y + C2 * ((_aw_y < (Zero - C1)) - (C1 < _aw_y)),
        reference=lambda in0, in1, s0, s1, imm2: (
            (in0 + in1)
            + imm2
            * (
                ((in0 + in1) < -s1).astype(np.float32)
                - ((in0 + in1) > s1).astype(np.float32)
            )
        ).astype(np.float32),
    ),
)

# out = |in0| + s0
ABS_SUB = _register_dve_op(
    "ABS_ADD_ANT",
    Spec(
        body=maxx(Src0, Zero - Src0) + C0,
        reference=lambda in0, in1, s0, s1, imm2: (np.abs(in0) + s0).astype(
            np.float32
        ),
    ),
)


def build_program(nc: bass.Bass, dbg: bool = False):
    def dbg_out(name, src_ap, shape, dtype=F32):
        if not dbg:
            return
        d = nc.dram_tensor("dbg_" + name, shape, dtype, kind="ExternalOutput").ap()
        nc.sync.dma_start(d[:], src_ap)

    csm_r = nc.dram_tensor("csm_r", [NCOIL, NX, NX], F32, kind="ExternalInput").ap()
    csm_i = nc.dram_tensor("csm_i", [NCOIL, NX, NX], F32, kind="ExternalInput").ap()
    kx_d = nc.dram_tensor("kx", [S], F32, kind="ExternalInput").ap()
    ky_d = nc.dram_tensor("ky", [S], F32, kind="ExternalInput").ap()
    tbl_d = nc.dram_tensor("tbl", [NPIX, ELEM], BF16, kind="ExternalInput").ap()
    idx_d = nc.dram_tensor("idx", [P, GIDX], I16, kind="ExternalInput").ap()
    w4_d = nc.dram_tensor("w4", [P, NX, 4], F32, kind="ExternalInput").ap()
    out_r = nc.dram_tensor("out_r", [NCOIL, S], F32, kind="ExternalOutput").ap()
    out_i = nc.dram_tensor("out_i", [NCOIL, S], F32, kind="ExternalOutput").ap()

    # ---------------- inline constants ----------------
    pvals = np.arange(P, dtype=np.float32)
    xc_d = nc.inline_tensor((pvals - 64.0).reshape(P, 1), name="c_xc").ap()
    yo8_d = nc.inline_tensor((8.0 * (np.arange(P) % 16)).astype(np.float32)
                             .reshape(P, 1), name="c_yo8").ap()
    half_pi_d = nc.inline_tensor(np.full((P, 1), math.pi / 2, np.float32),
                                 name="c_half_pi").ap()
    sel_np = (np.arange(P)[:, None] // 16 == np.arange(NCOIL)[None, :]).astype(
        np.float32)
    selpm_np = np.concatenate([sel_np, -sel_np], axis=1)  # [128, 16]: +sel | -sel
    selpm_d = nc.inline_tensor(selpm_np, name="c_selpm").ap()

    with tile.TileContext(nc) as tc, \
         tc.tile_pool(name="pp", bufs=1) as pp:

        # --- persistent constants / inputs ---
        idx16 = pp.tile([P, GIDX], I16)
        kxb = pp.tile([P, S], F32)
        nc.sync.dma_start(
            kxb[:], kx_d[:].rearrange("(p s) -> p s", p=1).to_broadcast([P, S]))
        kyb = pp.tile([P, S], F32)
        nc.sync.dma_start(
            kyb[:], ky_d[:].rearrange("(p s) -> p s", p=1).to_broadcast([P, S]))
        xc_col = pp.tile([P, 1], F32)
        nc.sync.dma_start(xc_col[:], xc_d[:])
        yo8 = pp.tile([P, 1], F32)
        nc.sync.dma_start(yo8[:], yo8_d[:])
        half_pi = pp.tile([P, 1], F32)
        nc.sync.dma_start(half_pi[:], half_pi_d[:])
        selpm32 = pp.tile([P, 2 * NCOIL], F32)
        nc.sync.dma_start(selpm32[:], selpm_d[:])
        selpm = pp.tile([P, 2 * NCOIL], BF16)
        nc.vector.tensor_copy(selpm[:], selpm32[:])

        # packed coil stationary: blocks [Cr | Ci | -Ci], col = c*16 + yo
        RA = pp.tile([P, YI, 3 * P], BF16)

        # --- pools (gp innermost so it can close after the warp) ---
        lp_ctx = tc.tile_pool(name="loop", bufs=1)
        lp = lp_ctx.__enter__()
        kp_ctx = tc.tile_pool(name="kr", bufs=1)
        kp = kp_ctx.__enter__()
        gp_pool_ctx = tc.tile_pool(name="gp", bufs=1)
        gp = gp_pool_ctx.__enter__()
        g8p = gp.tile([P, NX, ELEM], BF16)
        w4sb = gp.tile([P, NX, 4], F32)
        nc.sync.dma_start(w4sb[:], w4_d[:])
        csm_r_sb = gp.tile([P, NCOIL, NX], F32)
        nc.sync.dma_start(csm_r_sb[:], csm_r.rearrange("c x y -> x c y"))
        csm_i_sb = gp.tile([P, NCOIL, NX], F32)
        nc.sync.dma_start(csm_i_sb[:], csm_i.rearrange("c x y -> x c y"))

        # idx16 is loaded LAST on the sync queue: HWDGE executes FIFO, so the
        # gathers (which depend on idx16) cannot start stealing SDMA slots
        # until every other input DMA has landed.
        nc.sync.dma_start(idx16[:], idx_d[:])
        gsems = [nc.alloc_semaphore(f"gath_sem{q}") for q in range(4)]
        for h in range(NGATH):
            nc.gpsimd.dma_gather(
                out_ap=g8p[:, h * 8:(h + 1) * 8, :],
                in_ap=tbl_d[:],
                idxs_ap=idx16[:, h * 64:(h + 1) * 64],
                num_idxs=GIDX,
                num_idxs_reg=GIDX,
                elem_size=ELEM,
                queue_num=h % 4,
            ).then_inc(gsems[h % 4], 16)

        # ---------------- trig batches (emitted in pieces) ----------------
        trig = {}

        def make_trig(b):
            cs = slice(b * BW, (b + 1) * BW)
            st = {"m2": {}, "kits": [], "krts": []}

            def emit_yi(yi):
                m2 = st["m2"]
                kyc = st["kyc"]
                if yi > 0:
                    t = lp.tile([P, BW], F32, tag="m2c", bufs=2)
                    nc.vector._custom_dve(ADD_TT_WRAP, out=t[:],
                                          in0=m2[yi - 1][:],
                                          in1=kyc, s1=0.5, imm2=1.0)
                    m2[yi] = t
                kit = kp.tile([P, BW], BF16, tag=f"kit{yi}", bufs=2)
                nc.scalar.activation(kit[:], m2[yi][:], ACTF.Sin, scale=-TWO_PI)
                krt = kp.tile([P, BW], BF16, tag=f"krt{yi}", bufs=2)
                if yi < NABS_SC:
                    mabs = lp.tile([P, BW], F32, tag="mabs", bufs=1)
                    nc.scalar.activation(mabs[:], m2[yi][:], ACTF.Abs)
                    nc.scalar.activation(krt[:], mabs[:], ACTF.Sin,
                                         scale=-TWO_PI, bias=half_pi[:, 0:1])
                else:
                    mk = lp.tile([P, BW], F32, tag="mk", bufs=2)
                    nc.vector._custom_dve(ABS_SUB, out=mk[:], in0=m2[yi][:],
                                          s0=-0.25)
                    nc.scalar.activation(krt[:], mk[:], ACTF.Sin, scale=-TWO_PI)
                st["kits"].append(kit)
                st["krts"].append(krt)

            def piece0():
                kxc = kxb[:, cs]
                kyc = st["kyc"] = kyb[:, cs]
                m2o = lp.tile([P, BW], F32, tag="m2o", bufs=1)
                nc.vector._custom_dve(PHASE_WRAP, out=m2o[:], in0=kyc,
                                      in1=kyc, s0=yo8[:, 0:1], s1=0.0,
                                      imm2=MAGIC)
                mok = lp.tile([P, BW], F32, tag="mok", bufs=1)
                nc.vector._custom_dve(ABS_SUB, out=mok[:], in0=m2o[:], s0=-0.25)
                aic = kp.tile([P, BW], BF16, tag="aic", bufs=2)
                nc.scalar.activation(aic[:], m2o[:], ACTF.Sin, scale=-TWO_PI)
                arc = kp.tile([P, BW], BF16, tag="arc", bufs=2)
                nc.scalar.activation(arc[:], mok[:], ACTF.Sin, scale=-TWO_PI)
                m2a = lp.tile([P, BW], F32, tag="m2a", bufs=1)
                nc.vector._custom_dve(PHASE_WRAP, out=m2a[:], in0=kxc,
                                      in1=kyc, s0=xc_col[:, 0:1], s1=-64.0,
                                      imm2=MAGIC)
                st["m2"][0] = m2a
                emit_yi(0)
                trig[b] = (st["kits"], st["krts"], arc, aic)

            return [piece0] + [lambda yi=yi: emit_yi(yi) for yi in range(1, YI)]

        # ---------------- warp quarter: combine + pack ----------------
        def emit_quarter(q):
            ys = slice(32 * q, 32 * q + 32)
            for j in range(4):
                nc.vector.wait_ge(gsems[j], 16 * (q + 1))
            t8r = gp.tile([P, 32, 4], F32, tag="t8r", bufs=2)
            nc.vector.tensor_tensor(t8r[:], g8p[:, ys, 0:4], w4sb[:, ys, :],
                                    op=ALU.mult)
            warped_r = gp.tile([P, 32], F32, tag="wr", bufs=2)
            nc.vector.reduce_sum(warped_r[:], t8r[:], axis=mybir.AxisListType.X)
            t8i = gp.tile([P, 32, 4], F32, tag="t8i", bufs=2)
            nc.vector.tensor_tensor(t8i[:], g8p[:, ys, 4:8], w4sb[:, ys, :],
                                    op=ALU.mult)
            warped_i = gp.tile([P, 32], F32, tag="wi", bufs=2)
            nc.vector.reduce_sum(warped_i[:], t8i[:], axis=mybir.AxisListType.X)

            wr_b = warped_r[:].rearrange("p (c y) -> p c y", c=1).to_broadcast(
                [P, NCOIL, 32])
            wi_b = warped_i[:].rearrange("p (c y) -> p c y", c=1).to_broadcast(
                [P, NCOIL, 32])
            csr = csm_r_sb[:, :, ys]
            csi = csm_i_sb[:, :, ys]

            # RA views for this quarter: [p, c, yo(4), yi(8)]
            ra5 = RA[:].rearrange("p yi (b c yo) -> p b c yo yi", b=3, c=NCOIL)
            cr_v = ra5[:, 0, :, 4 * q:4 * q + 4, :]
            ci_v = ra5[:, 1, :, 4 * q:4 * q + 4, :]
            cin_v = ra5[:, 2, :, 4 * q:4 * q + 4, :]

            def as4(t):
                return t.rearrange("p c (yo yi) -> p c yo yi", yi=YI)

            tt1 = gp.tile([P, NCOIL, 32], F32, tag="tt1", bufs=2)
            nc.vector.tensor_tensor(tt1[:], csr, wr_b, op=ALU.mult)
            tt2 = gp.tile([P, NCOIL, 32], F32, tag="tt2", bufs=2)
            nc.vector.tensor_tensor(tt2[:], csi, wi_b, op=ALU.mult)
            nc.vector.tensor_tensor(cr_v, as4(tt1[:]), as4(tt2[:]),
                                    op=ALU.subtract)
            tt3 = gp.tile([P, NCOIL, 32], F32, tag="tt1", bufs=2)
            nc.vector.tensor_tensor(tt3[:], csr, wi_b, op=ALU.mult)
            tt4 = gp.tile([P, NCOIL, 32], F32, tag="tt2", bufs=2)
            nc.vector.tensor_tensor(tt4[:], csi, wr_b, op=ALU.mult)
            cit = gp.tile([P, NCOIL, 32], F32, tag="cit", bufs=2)
            nc.vector.tensor_tensor(cit[:], tt3[:], tt4[:], op=ALU.add)
            nc.vector.tensor_copy(ci_v, as4(cit[:]))
            nc.vector.tensor_scalar(cin_v, as4(cit[:]), -1.0, None, op0=ALU.mult)

        # ---------------- MM chunk ----------------
        ps_ctx = tc.tile_pool(name="ps", bufs=1, space="PSUM")
        ps = ps_ctx.__enter__()
        pso_ctx = tc.tile_pool(name="pso", bufs=1, space="PSUM")
        pso = pso_ctx.__enter__()

        live = {}

        def emit_mains(ch):
            b, half = divmod(ch, 2)
            sl = slice(half * CH, (half + 1) * CH)
            kits, krts, arc, aic = trig[b]
            Pr = ps.tile([P, CH], F32, tag="Pr", bufs=2)
            Pi = ps.tile([P, CH], F32, tag="Pi", bufs=2)
            for yi in range(YI):
                st, sp = (yi == 0), (yi == YI - 1)
                krt_s = krts[yi][:, sl]
                kit_s = kits[yi][:, sl]
                nc.tensor.matmul(Pr[:], RA[:, yi, 0:128], krt_s,
                                 start=st, stop=False)
                nc.tensor.matmul(Pi[:], RA[:, yi, 0:128], kit_s,
                                 start=st, stop=False)
                nc.tensor.matmul(Pr[:], RA[:, yi, 256:384], kit_s,
                                 start=False, stop=sp)
                nc.tensor.matmul(Pi[:], RA[:, yi, 128:256], krt_s,
                                 start=False, stop=sp)
            live[ch] = (Pr, Pi, arc, aic, sl)

        def emit_post(ch):
            c0 = ch * CH
            Pr, Pi, arc, aic, sl = live.pop(ch)
            q1 = lp.tile([P, CH], BF16, tag="q1", bufs=2)
            nc.vector.tensor_tensor(q1[:], Pr[:], arc[:, sl], op=ALU.mult)
            q2 = lp.tile([P, CH], BF16, tag="q2", bufs=2)
            nc.vector.tensor_tensor(q2[:], Pi[:], aic[:, sl], op=ALU.mult)
            eng3 = nc.gpsimd if GP_OUTER >= 1 else nc.vector
            eng4 = nc.gpsimd if GP_OUTER >= 2 else nc.vector
            q3 = lp.tile([P, CH], BF16, tag="q3", bufs=2)
            eng3.tensor_tensor(q3[:], Pi[:], arc[:, sl], op=ALU.mult)
            q4 = lp.tile([P, CH], BF16, tag="q4", bufs=2)
            eng4.tensor_tensor(q4[:], Pr[:], aic[:, sl], op=ALU.mult)

            SP, SM = selpm[:, 0:NCOIL], selpm[:, NCOIL:2 * NCOIL]
            por = pso.tile([NCOIL, CH], F32, tag="por", bufs=2)
            nc.tensor.matmul(por[:], SP, q1[:], start=True, stop=False)
            nc.tensor.matmul(por[:], SM, q2[:], start=False, stop=True)
            poi = pso.tile([NCOIL, CH], F32, tag="poi", bufs=2)
            nc.tensor.matmul(poi[:], SP, q3[:], start=True, stop=False)
            nc.tensor.matmul(poi[:], SP, q4[:], start=False, stop=True)
            osr = lp.tile([NCOIL, CH], F32, tag="osr", bufs=1)
            nc.scalar.copy(osr[:], por[:])
            osi = lp.tile([NCOIL, CH], F32, tag="osi", bufs=1)
            nc.scalar.copy(osi[:], poi[:])
            nc.sync.dma_start(out_r[:, c0:c0 + CH], osr[:])
            nc.sync.dma_start(out_i[:, c0:c0 + CH], osi[:])

        # ---------------- emission schedule ----------------
        for p in make_trig(0):
            p()
        for q in range(4):
            emit_quarter(q)
        dbg_out("RA", RA[:].rearrange("p yi c -> p (yi c)"), [P, YI * 3 * P],
                BF16)
        gp_pool_ctx.__exit__(None, None, None)
        for p in make_trig(1):
            p()

        # interleave trig batches 2/3 into the MM loop so the DVE queue never
        # blocks the selector matmuls: b2 pieces land after posts 1-3, b3
        # after posts 3-5.
        t2 = make_trig(2)
        t3 = make_trig(3)
        pieces = {0: t2[0:3], 1: t2[3:6], 2: t2[6:8] + t3[0:1],
                  3: t3[1:4], 4: t3[4:8]}

        for ch in range(NCHUNK):
            emit_mains(ch)
            if ch > 0:
                emit_post(ch - 1)
                for p in pieces.get(ch - 1, []):
                    p()
        emit_post(NCHUNK - 1)

        pso_ctx.__exit__(None, None, None)
        ps_ctx.__exit__(None, None, None)
        kp_ctx.__exit__(None, None, None)
        lp_ctx.__exit__(None, None, None)


_COMPILED = {}


def _get_nc(dbg: bool = False):
    key = ("nc", dbg)
    if key not in _COMPILED:
        nc = bacc.Bacc("TRN2", debug=False, num_swdge_queues=4)
        build_program(nc, dbg=dbg)
        nc.compile()
        _COMPILED[key] = nc
    return _COMPILED[key]


# slot g = 16*j + (p%16)  <->  output pixel (x = g%128, y = g//128);
# gather h covers slots [1024h, 1024(h+1)) -> partitions x, columns y.
_Jg = np.arange(GIDX)[None, :]
_Pg = np.arange(P)[:, None]
_G = 16 * _Jg + (_Pg % 16)            # [128, 1024]
_XG = (_G % 128).astype(np.int64)
_YG = (_G // 128).astype(np.int64)
_BF16 = ml_dtypes.bfloat16


def _build_tables(image_r, image_i, flow):
    """Per-timepoint: corner table (bf16, row y0*128+x0), idx16, weights."""
    ir = np.ascontiguousarray(image_r, np.float32)
    ii = np.ascontiguousarray(image_i, np.float32)
    irT, iiT = ir.T, ii.T                     # [y, x]
    y1 = np.minimum(np.arange(NX) + 1, NX - 1)
    x1 = np.minimum(np.arange(NX) + 1, NX - 1)
    tables = []
    for t in range(NT):
        f0 = np.asarray(flow[:, :, 0, t], np.float32)
        f1 = np.asarray(flow[:, :, 1, t], np.float32)
        # float32 math mirrors the jax reference exactly
        xg = np.arange(NX, dtype=np.float32)[:, None]
        yg = np.arange(NX, dtype=np.float32)[None, :]
        cx = np.clip(xg + f0, np.float32(0.0), np.float32(NX - 1))
        cy = np.clip(yg + f1, np.float32(0.0), np.float32(NX - 1))
        x0 = np.floor(cx)
        y0 = np.floor(cy)
        wx = (cx - x0).astype(np.float32)     # [x, y]
        wy = (cy - y0).astype(np.float32)
        w4 = np.stack([(1 - wx) * (1 - wy), (1 - wx) * wy,
                       wx * (1 - wy), wx * wy], axis=-1).astype(np.float32)
        x0i = x0.astype(np.int64)
        y0i = y0.astype(np.int64)
        idxv = (y0i * NX + x0i).astype(np.int16)      # [x, y]
        idx16 = idxv[_XG, _YG]                        # wrapped gather layout

        tbl = np.zeros((NX, NX, ELEM), dtype=_BF16)
        tbl[:, :, 0] = irT
        tbl[:, :, 1] = irT[y1, :]
        tbl[:, :, 2] = irT[:, x1]
        tbl[:, :, 3] = irT[y1][:, x1]
        tbl[:, :, 4] = iiT
        tbl[:, :, 5] = iiT[y1, :]
        tbl[:, :, 6] = iiT[:, x1]
        tbl[:, :, 7] = iiT[y1][:, x1]
        tables.append((tbl.reshape(NPIX, ELEM), idx16, w4))
    return tables


def make_in_maps(image_r, image_i, csm_r, csm_i, traj, dcf, flow):
    del dcf  # unused by the operator
    tables = _build_tables(image_r, image_i, flow)
    csm_r = np.ascontiguousarray(csm_r, np.float32)
    csm_i = np.ascontiguousarray(csm_i, np.float32)
    in_maps = []
    for core in range(8):
        t, h = divmod(core, 2)
        sl = slice(h * S, (h + 1) * S)
        tbl, idx16, w4 = tables[t]
        in_maps.append({
            "csm_r": csm_r,
            "csm_i": csm_i,
            "kx": np.ascontiguousarray(traj[sl, 0, t], np.float32),
            "ky": np.ascontiguousarray(traj[sl, 1, t], np.float32),
            "tbl": np.ascontiguousarray(tbl),
            "idx": np.ascontiguousarray(idx16),
            "w4": np.ascontiguousarray(w4),
        })
    return in_maps


def combine_outputs(results):
    out = np.zeros((NCOIL, NS), np.complex64)
    for core, res in enumerate(results):
        t, h = divmod(core, 2)
        sl = slice(h * S, (h + 1) * S)
        out[:, sl] += res["out_r"].astype(np.complex64) + 1j * res["out_i"].astype(
            np.complex64)
    return out


def kernel(**inputs) -> np.ndarray:
    from concourse.bass_utils import run_bass_kernel_spmd

    nc = _get_nc()
    in_maps = make_in_maps(**inputs)
    res = run_bass_kernel_spmd(nc, in_maps, core_ids=list(range(8)))
    return combine_outputs(res.results)


# revision 17
# speedup vs baseline: 1.7068x; 1.0051x over previous
"""Batchelor GPU-NUFFT forward operator on 8 Trainium2 NeuronCores.

Math (per timepoint t):
    warped  = bilinear_warp(image, flow[..., t])
    coil    = csm * warped                                  [Nc,Nx,Ny]
    out_t[c,s] = sum_{x,y} coil[c,x,y] exp(-2pi i (kx_s (x-64) + ky_s (y-64)))
    out     = sum_t out_t                                   [Nc,NS] complex64

Sharding: 8 cores = 4 timepoints x 2 sample-halves (4096 samples each).
Host unshard: sum the 4 timepoint partials per half, concat halves.

Device pipeline (per core):
  * warp: host provides the bf16 corner table (DRAM, row (y0*128+x0) holds the
    4 bilinear corners of real+imag), int16 gather indices in the SWDGE
    wrapped layout, and the 4 bilinear weight planes. 16 dma_gather ops land
    the corners directly in [x, y] layout (slot i = y*128 + x); the combine
    and the coil pack run per 4-gather quarter in the gather shadow.
  * NUFFT: Khatri-Rao split y = yo*8 + yi. Per 512-sample chunk, 32
    accumulating bf16 matmuls build PSUM partials Pr = Re(sum coil e^{-iA}),
    Pi = Im(...) directly (stationary blocks Cr | Ci | -Ci make the +- signs
    accumulate in PSUM). The outer phase e^{-iB} is 4 elementwise products,
    folded to 8 coils by +-selector matmuls.
  * trig: phases are range-reduced with custom DVE ops (PHASE_WRAP fuses the
    a0 = kx*(x-64) - 64*ky wrap to one op; ADD_TT_WRAP fuses each chain step
    m2_yi = wrap(m2_parent + ky2^j) with a log-depth parent tree; ABS_SUB
    preps cos args as |m|-1/4 since the ACT Sin spline is only valid on
    [-pi, pi]). ky2/ky4 = wrap(2ky), wrap(4ky) come from the host. All four
    1024-wide trig batches are emitted ahead of the MM loop so the Scalar
    engine streams Sin evaluations while the gather runs.
"""

import sys

if "/opt/trn_rl_repo" not in sys.path:
    sys.path.insert(0, "/opt/trn_rl_repo")

import math

import numpy as np
import ml_dtypes

import concourse.bass as bass
import concourse.tile as tile
from concourse import bacc
from concourse import mybir
from concourse import dve_ops
from concourse.dve_spec import Spec, Src0, Src1, C0, C1, C2, Zero, maxx

P = 128
NX = 128
NCOIL = 8
NS = 8192
NT = 4
S = 4096           # samples per core (half of NS)
CH = 512           # samples per MM chunk (PSUM bank width)
NCHUNK = S // CH   # 8
BW = 1024          # trig batch width (2 chunks)
NBATCH = S // BW   # 4
YI = 8
YO = 16
NPIX = NX * NX
NGATH = 16
GIDX = NPIX // NGATH   # 1024 indices per gather
ELEM = 128             # bf16 elements per table row = 256 bytes
NABS_SC = 5            # yi < NABS_SC: cos-prep via scalar Abs; else DVE ABS_SUB
GP_OUTER = 0           # gpsimd cannot read PSUM: outer products stay on DVE

F32 = mybir.dt.float32
BF16 = mybir.dt.bfloat16
I16 = mybir.dt.int16
TWO_PI = float(2.0 * math.pi)
MAGIC = 12582912.0  # 1.5*2^23: (x + M) - M == round-to-nearest(x) for f32
ALU = mybir.AluOpType
ACTF = mybir.ActivationFunctionType


# ---------------- custom DVE ops ----------------
def _register_dve_op(name, spec):
    if name in dve_ops._SUB_OPCODE_FOR_NAME:
        for op in dve_ops.OPS:
            if op.name == name:
                return op
        raise RuntimeError(name)
    shas = {}
    for ver in ("v3", "v4"):
        uops = dve_ops.lower(spec, ver=ver)
        shas[ver] = dve_ops.DveOpSpec(
            name=name, opcode=0, uops=uops, rd1_en=dve_ops.has_src1(spec)
        ).sha(ver)
    op = dve_ops.DveOp(name, spec, subdim=False, uops_sha=shas)
    dve_ops.OPS.append(op)
    dve_ops._SUB_OPCODE_FOR_NAME[name] = (
        dve_ops._CUSTOM_DVE_ROW_BASE + len(dve_ops.OPS) - 1
    )
    dve_ops.CUSTOM_DVE_SPECS[name] = spec
    return op


def _wrap_np(v):
    return (v - np.round(v)).astype(np.float32)


# out = m - round(m), m = in0*s0 + in1*s1  (s0 may be a [P,1] AP)
_pw_m = Src0 * C0 + Src1 * C1
_pw_r = (_pw_m + C2) - C2
PHASE_WRAP = _register_dve_op(
    "PHASE_WRAP_ANT",
    Spec(
        body=_pw_m - _pw_r,
        reference=lambda in0, in1, s0, s1, imm2: (
            (in0 * s0 + in1 * s1)
            - (((in0 * s0 + in1 * s1) + imm2) - imm2)
        ).astype(np.float32),
    ),
)

# out = y - ((y > .5) - (y < -.5)), y = in0 + in1 : one-period wrap of a sum
_aw_y = Src0 + Src1
ADD_TT_WRAP = _register_dve_op(
    "ADD_TT_WRAP_ANT",
    Spec(
        body=_aw_y + C2 * ((_aw_y < (Zero - C1)) - (C1 < _aw_y)),
        reference=lambda in0, in1, s0, s1, imm2: (
            (in0 + in1)
            + imm2
            * (
                ((in0 + in1) < -s1).astype(np.float32)
                - ((in0 + in1) > s1).astype(np.float32)
            )
        ).astype(np.float32),
    ),
)

# out = |in0| + s0
ABS_SUB = _register_dve_op(
    "ABS_ADD_ANT",
    Spec(
        body=maxx(Src0, Zero - Src0) + C0,
        reference=lambda in0, in1, s0, s1, imm2: (np.abs(in0) + s0).astype(
            np.float32
        ),
    ),
)


def build_program(nc: bass.Bass, dbg: bool = False):
    def dbg_out(name, src_ap, shape, dtype=F32):
        if not dbg:
            return
        d = nc.dram_tensor("dbg_" + name, shape, dtype, kind="ExternalOutput").ap()
        nc.sync.dma_start(d[:], src_ap)

    csm_r = nc.dram_tensor("csm_r", [NCOIL, NX, NX], F32, kind="ExternalInput").ap()
    csm_i = nc.dram_tensor("csm_i", [NCOIL, NX, NX], F32, kind="ExternalInput").ap()
    kx_d = nc.dram_tensor("kx", [S], F32, kind="ExternalInput").ap()
    ky_d = nc.dram_tensor("ky", [S], F32, kind="ExternalInput").ap()
    tbl_d = nc.dram_tensor("tbl", [NPIX, ELEM], BF16, kind="ExternalInput").ap()
    idx_d = nc.dram_tensor("idx", [P, GIDX], I16, kind="ExternalInput").ap()
    w4_d = nc.dram_tensor("w4", [P, NX, 4], F32, kind="ExternalInput").ap()
    out_r = nc.dram_tensor("out_r", [NCOIL, S], F32, kind="ExternalOutput").ap()
    out_i = nc.dram_tensor("out_i", [NCOIL, S], F32, kind="ExternalOutput").ap()

    # ---------------- inline constants ----------------
    pvals = np.arange(P, dtype=np.float32)
    xc_d = nc.inline_tensor((pvals - 64.0).reshape(P, 1), name="c_xc").ap()
    yo8_d = nc.inline_tensor((8.0 * (np.arange(P) % 16)).astype(np.float32)
                             .reshape(P, 1), name="c_yo8").ap()
    half_pi_d = nc.inline_tensor(np.full((P, 1), math.pi / 2, np.float32),
                                 name="c_half_pi").ap()
    sel_np = (np.arange(P)[:, None] // 16 == np.arange(NCOIL)[None, :]).astype(
        np.float32)
    selpm_np = np.concatenate([sel_np, -sel_np], axis=1)  # [128, 16]: +sel | -sel
    selpm_d = nc.inline_tensor(selpm_np, name="c_selpm").ap()

    with tile.TileContext(nc) as tc, \
         tc.tile_pool(name="pp", bufs=1) as pp:

        # --- persistent constants / inputs ---
        idx16 = pp.tile([P, GIDX], I16)
        H = S // 2
        kxb = pp.tile([P, S], F32)
        nc.sync.dma_start(
            kxb[:, 0:H],
            kx_d[0:H].rearrange("(p s) -> p s", p=1).to_broadcast([P, H]))
        kyb = pp.tile([P, S], F32)
        nc.sync.dma_start(
            kyb[:, 0:H],
            ky_d[0:H].rearrange("(p s) -> p s", p=1).to_broadcast([P, H]))
        xc_col = pp.tile([P, 1], F32)
        nc.sync.dma_start(xc_col[:], xc_d[:])
        yo8 = pp.tile([P, 1], F32)
        nc.sync.dma_start(yo8[:], yo8_d[:])
        half_pi = pp.tile([P, 1], F32)
        nc.sync.dma_start(half_pi[:], half_pi_d[:])
        selpm32 = pp.tile([P, 2 * NCOIL], F32)
        nc.sync.dma_start(selpm32[:], selpm_d[:])
        selpm = pp.tile([P, 2 * NCOIL], BF16)
        nc.vector.tensor_copy(selpm[:], selpm32[:])

        # packed coil stationary: blocks [Cr | Ci | -Ci], col = c*16 + yo
        RA = pp.tile([P, YI, 3 * P], BF16)

        # --- pools (gp innermost so it can close after the warp) ---
        lp_ctx = tc.tile_pool(name="loop", bufs=1)
        lp = lp_ctx.__enter__()
        kp_ctx = tc.tile_pool(name="kr", bufs=1)
        kp = kp_ctx.__enter__()
        gp_pool_ctx = tc.tile_pool(name="gp", bufs=1)
        gp = gp_pool_ctx.__enter__()
        g8p = gp.tile([P, NX, ELEM], BF16)
        w4sb = gp.tile([P, NX, 4], F32)
        nc.sync.dma_start(w4sb[:], w4_d[:])
        csm_r_sb = gp.tile([P, NCOIL, NX], F32)
        nc.sync.dma_start(csm_r_sb[:], csm_r.rearrange("c x y -> x c y"))
        csm_i_sb = gp.tile([P, NCOIL, NX], F32)
        nc.sync.dma_start(csm_i_sb[:], csm_i.rearrange("c x y -> x c y"))

        # idx16 is loaded LAST on the sync queue: HWDGE executes FIFO, so the
        # gathers (which depend on idx16) cannot start stealing SDMA slots
        # until every other input DMA has landed.
        nc.sync.dma_start(idx16[:], idx_d[:])
        nc.sync.dma_start(
            kxb[:, H:S],
            kx_d[H:S].rearrange("(p s) -> p s", p=1).to_broadcast([P, S - H]))
        nc.sync.dma_start(
            kyb[:, H:S],
            ky_d[H:S].rearrange("(p s) -> p s", p=1).to_broadcast([P, S - H]))
        gsems = [nc.alloc_semaphore(f"gath_sem{q}") for q in range(4)]
        for h in range(NGATH):
            nc.gpsimd.dma_gather(
                out_ap=g8p[:, h * 8:(h + 1) * 8, :],
                in_ap=tbl_d[:],
                idxs_ap=idx16[:, h * 64:(h + 1) * 64],
                num_idxs=GIDX,
                num_idxs_reg=GIDX,
                elem_size=ELEM,
                queue_num=h % 4,
            ).then_inc(gsems[h % 4], 16)

        # ---------------- trig batches (emitted in pieces) ----------------
        trig = {}

        def make_trig(b):
            cs = slice(b * BW, (b + 1) * BW)
            st = {"m2": {}, "kits": [], "krts": []}

            def emit_yi(yi):
                m2 = st["m2"]
                kyc = st["kyc"]
                if yi > 0:
                    t = lp.tile([P, BW], F32, tag="m2c", bufs=2)
                    nc.vector._custom_dve(ADD_TT_WRAP, out=t[:],
                                          in0=m2[yi - 1][:],
                                          in1=kyc, s1=0.5, imm2=1.0)
                    m2[yi] = t
                kit = kp.tile([P, BW], BF16, tag=f"kit{yi}", bufs=2)
                nc.scalar.activation(kit[:], m2[yi][:], ACTF.Sin, scale=-TWO_PI)
                krt = kp.tile([P, BW], BF16, tag=f"krt{yi}", bufs=2)
                if yi < NABS_SC:
                    mabs = lp.tile([P, BW], F32, tag="mabs", bufs=1)
                    nc.scalar.activation(mabs[:], m2[yi][:], ACTF.Abs)
                    nc.scalar.activation(krt[:], mabs[:], ACTF.Sin,
                                         scale=-TWO_PI, bias=half_pi[:, 0:1])
                else:
                    mk = lp.tile([P, BW], F32, tag="mk", bufs=2)
                    nc.vector._custom_dve(ABS_SUB, out=mk[:], in0=m2[yi][:],
                                          s0=-0.25)
                    nc.scalar.activation(krt[:], mk[:], ACTF.Sin, scale=-TWO_PI)
                st["kits"].append(kit)
                st["krts"].append(krt)

            def piece0():
                kxc = kxb[:, cs]
                kyc = st["kyc"] = kyb[:, cs]
                m2o = lp.tile([P, BW], F32, tag="m2o", bufs=1)
                nc.vector._custom_dve(PHASE_WRAP, out=m2o[:], in0=kyc,
                                      in1=kyc, s0=yo8[:, 0:1], s1=0.0,
                                      imm2=MAGIC)
                mok = lp.tile([P, BW], F32, tag="mok", bufs=1)
                nc.vector._custom_dve(ABS_SUB, out=mok[:], in0=m2o[:], s0=-0.25)
                aic = kp.tile([P, BW], BF16, tag="aic", bufs=2)
                nc.scalar.activation(aic[:], m2o[:], ACTF.Sin, scale=-TWO_PI)
                arc = kp.tile([P, BW], BF16, tag="arc", bufs=2)
                nc.scalar.activation(arc[:], mok[:], ACTF.Sin, scale=-TWO_PI)
                m2a = lp.tile([P, BW], F32, tag="m2a", bufs=1)
                nc.vector._custom_dve(PHASE_WRAP, out=m2a[:], in0=kxc,
                                      in1=kyc, s0=xc_col[:, 0:1], s1=-64.0,
                                      imm2=MAGIC)
                st["m2"][0] = m2a
                emit_yi(0)
                trig[b] = (st["kits"], st["krts"], arc, aic)

            return [piece0] + [lambda yi=yi: emit_yi(yi) for yi in range(1, YI)]

        # ---------------- warp quarter: combine + pack ----------------
        def emit_quarter(q):
            ys = slice(32 * q, 32 * q + 32)
            for j in range(4):
                nc.vector.wait_ge(gsems[j], 16 * (q + 1))
            t8r = gp.tile([P, 32, 4], F32, tag="t8r", bufs=2)
            nc.vector.tensor_tensor(t8r[:], g8p[:, ys, 0:4], w4sb[:, ys, :],
                                    op=ALU.mult)
            warped_r = gp.tile([P, 32], F32, tag="wr", bufs=2)
            nc.vector.reduce_sum(warped_r[:], t8r[:], axis=mybir.AxisListType.X)
            t8i = gp.tile([P, 32, 4], F32, tag="t8i", bufs=2)
            nc.vector.tensor_tensor(t8i[:], g8p[:, ys, 4:8], w4sb[:, ys, :],
                                    op=ALU.mult)
            warped_i = gp.tile([P, 32], F32, tag="wi", bufs=2)
            nc.vector.reduce_sum(warped_i[:], t8i[:], axis=mybir.AxisListType.X)

            wr_b = warped_r[:].rearrange("p (c y) -> p c y", c=1).to_broadcast(
                [P, NCOIL, 32])
            wi_b = warped_i[:].rearrange("p (c y) -> p c y", c=1).to_broadcast(
                [P, NCOIL, 32])
            csr = csm_r_sb[:, :, ys]
            csi = csm_i_sb[:, :, ys]

            # RA views for this quarter: [p, c, yo(4), yi(8)]
            ra5 = RA[:].rearrange("p yi (b c yo) -> p b c yo yi", b=3, c=NCOIL)
            cr_v = ra5[:, 0, :, 4 * q:4 * q + 4, :]
            ci_v = ra5[:, 1, :, 4 * q:4 * q + 4, :]
            cin_v = ra5[:, 2, :, 4 * q:4 * q + 4, :]

            def as4(t):
                return t.rearrange("p c (yo yi) -> p c yo yi", yi=YI)

            tt1 = gp.tile([P, NCOIL, 32], F32, tag="tt1", bufs=2)
            nc.vector.tensor_tensor(tt1[:], csr, wr_b, op=ALU.mult)
            tt2 = gp.tile([P, NCOIL, 32], F32, tag="tt2", bufs=2)
            nc.vector.tensor_tensor(tt2[:], csi, wi_b, op=ALU.mult)
            nc.vector.tensor_tensor(cr_v, as4(tt1[:]), as4(tt2[:]),
                                    op=ALU.subtract)
            tt3 = gp.tile([P, NCOIL, 32], F32, tag="tt1", bufs=2)
            nc.vector.tensor_tensor(tt3[:], csr, wi_b, op=ALU.mult)
            tt4 = gp.tile([P, NCOIL, 32], F32, tag="tt2", bufs=2)
            nc.vector.tensor_tensor(tt4[:], csi, wr_b, op=ALU.mult)
            cit = gp.tile([P, NCOIL, 32], F32, tag="cit", bufs=2)
            nc.vector.tensor_tensor(cit[:], tt3[:], tt4[:], op=ALU.add)
            nc.vector.tensor_copy(ci_v, as4(cit[:]))
            nc.vector.tensor_scalar(cin_v, as4(cit[:]), -1.0, None, op0=ALU.mult)

        # ---------------- MM chunk ----------------
        ps_ctx = tc.tile_pool(name="ps", bufs=1, space="PSUM")
        ps = ps_ctx.__enter__()
        pso_ctx = tc.tile_pool(name="pso", bufs=1, space="PSUM")
        pso = pso_ctx.__enter__()

        live = {}

        def emit_mains(ch):
            b, half = divmod(ch, 2)
            sl = slice(half * CH, (half + 1) * CH)
            kits, krts, arc, aic = trig[b]
            Pr = ps.tile([P, CH], F32, tag="Pr", bufs=3)
            Pi = ps.tile([P, CH], F32, tag="Pi", bufs=3)
            for yi in range(YI):
                st, sp = (yi == 0), (yi == YI - 1)
                krt_s = krts[yi][:, sl]
                kit_s = kits[yi][:, sl]
                nc.tensor.matmul(Pr[:], RA[:, yi, 0:128], krt_s,
                                 start=st, stop=False)
                nc.tensor.matmul(Pi[:], RA[:, yi, 0:128], kit_s,
                                 start=st, stop=False)
                nc.tensor.matmul(Pr[:], RA[:, yi, 256:384], kit_s,
                                 start=False, stop=sp)
                nc.tensor.matmul(Pi[:], RA[:, yi, 128:256], krt_s,
                                 start=False, stop=sp)
            live[ch] = (Pr, Pi, arc, aic, sl)

        def emit_post(ch):
            c0 = ch * CH
            Pr, Pi, arc, aic, sl = live.pop(ch)
            q1 = lp.tile([P, CH], BF16, tag="q1", bufs=2)
            nc.vector.tensor_tensor(q1[:], Pr[:], arc[:, sl], op=ALU.mult)
            q2 = lp.tile([P, CH], BF16, tag="q2", bufs=2)
            nc.vector.tensor_tensor(q2[:], Pi[:], aic[:, sl], op=ALU.mult)
            eng3 = nc.gpsimd if GP_OUTER >= 1 else nc.vector
            eng4 = nc.gpsimd if GP_OUTER >= 2 else nc.vector
            q3 = lp.tile([P, CH], BF16, tag="q3", bufs=2)
            eng3.tensor_tensor(q3[:], Pi[:], arc[:, sl], op=ALU.mult)
            q4 = lp.tile([P, CH], BF16, tag="q4", bufs=2)
            eng4.tensor_tensor(q4[:], Pr[:], aic[:, sl], op=ALU.mult)

            SP, SM = selpm[:, 0:NCOIL], selpm[:, NCOIL:2 * NCOIL]
            po = pso.tile([32 + NCOIL, CH], F32, tag="po", bufs=2)
            nc.tensor.matmul(po[0:NCOIL], SP, q1[:], start=True, stop=False)
            nc.tensor.matmul(po[0:NCOIL], SM, q2[:], start=False, stop=True)
            nc.tensor.matmul(po[32:32 + NCOIL], SP, q3[:], start=True,
                             stop=False)
            nc.tensor.matmul(po[32:32 + NCOIL], SP, q4[:], start=False,
                             stop=True)
            ost = lp.tile([32 + NCOIL, CH], F32, tag="ost", bufs=2)
            nc.scalar.copy(ost[:], po[:])
            nc.sync.dma_start(out_r[:, c0:c0 + CH], ost[0:NCOIL])
            nc.sync.dma_start(out_i[:, c0:c0 + CH], ost[32:32 + NCOIL])

        # ---------------- emission schedule ----------------
        for p in make_trig(0):
            p()
        for q in range(4):
            emit_quarter(q)
        dbg_out("RA", RA[:].rearrange("p yi c -> p (yi c)"), [P, YI * 3 * P],
                BF16)
        gp_pool_ctx.__exit__(None, None, None)
        for p in make_trig(1):
            p()

        # interleave trig batches 2/3 into the MM loop so the DVE queue never
        # blocks the selector matmuls: b2 pieces land after posts 1-3, b3
        # after posts 3-5.
        t2 = make_trig(2)
        t3 = make_trig(3)
        pieces = {0: t2[0:3], 1: t2[3:6], 2: t2[6:8] + t3[0:1],
                  3: t3[1:4], 4: t3[4:8]}

        for ch in range(NCHUNK):
            emit_mains(ch)
            if ch > 0:
                emit_post(ch - 1)
                for p in pieces.get(ch - 1, []):
                    p()
        emit_post(NCHUNK - 1)

        pso_ctx.__exit__(None, None, None)
        ps_ctx.__exit__(None, None, None)
        kp_ctx.__exit__(None, None, None)
        lp_ctx.__exit__(None, None, None)


_COMPILED = {}


def _get_nc(dbg: bool = False):
    key = ("nc", dbg)
    if key not in _COMPILED:
        nc = bacc.Bacc("TRN2", debug=False, num_swdge_queues=4)
        build_program(nc, dbg=dbg)
        nc.compile()
        _COMPILED[key] = nc
    return _COMPILED[key]


# slot g = 16*j + (p%16)  <->  output pixel (x = g%128, y = g//128);
# gather h covers slots [1024h, 1024(h+1)) -> partitions x, columns y.
_Jg = np.arange(GIDX)[None, :]
_Pg = np.arange(P)[:, None]
_G = 16 * _Jg + (_Pg % 16)            # [128, 1024]
_XG = (_G % 128).astype(np.int64)
_YG = (_G // 128).astype(np.int64)
_BF16 = ml_dtypes.bfloat16


def _build_tables(image_r, image_i, flow):
    """Per-timepoint: corner table (bf16, row y0*128+x0), idx16, weights."""
    ir = np.ascontiguousarray(image_r, np.float32)
    ii = np.ascontiguousarray(image_i, np.float32)
    irT, iiT = ir.T, ii.T                     # [y, x]
    y1 = np.minimum(np.arange(NX) + 1, NX - 1)
    x1 = np.minimum(np.arange(NX) + 1, NX - 1)
    tables = []
    for t in range(NT):
        f0 = np.asarray(flow[:, :, 0, t], np.float32)
        f1 = np.asarray(flow[:, :, 1, t], np.float32)
        # float32 math mirrors the jax reference exactly
        xg = np.arange(NX, dtype=np.float32)[:, None]
        yg = np.arange(NX, dtype=np.float32)[None, :]
        cx = np.clip(xg + f0, np.float32(0.0), np.float32(NX - 1))
        cy = np.clip(yg + f1, np.float32(0.0), np.float32(NX - 1))
        x0 = np.floor(cx)
        y0 = np.floor(cy)
        wx = (cx - x0).astype(np.float32)     # [x, y]
        wy = (cy - y0).astype(np.float32)
        w4 = np.stack([(1 - wx) * (1 - wy), (1 - wx) * wy,
                       wx * (1 - wy), wx * wy], axis=-1).astype(np.float32)
        x0i = x0.astype(np.int64)
        y0i = y0.astype(np.int64)
        idxv = (y0i * NX + x0i).astype(np.int16)      # [x, y]
        idx16 = idxv[_XG, _YG]                        # wrapped gather layout

        tbl = np.zeros((NX, NX, ELEM), dtype=_BF16)
        tbl[:, :, 0] = irT
        tbl[:, :, 1] = irT[y1, :]
        tbl[:, :, 2] = irT[:, x1]
        tbl[:, :, 3] = irT[y1][:, x1]
        tbl[:, :, 4] = iiT
        tbl[:, :, 5] = iiT[y1, :]
        tbl[:, :, 6] = iiT[:, x1]
        tbl[:, :, 7] = iiT[y1][:, x1]
        tables.append((tbl.reshape(NPIX, ELEM), idx16, w4))
    return tables


def make_in_maps(image_r, image_i, csm_r, csm_i, traj, dcf, flow):
    del dcf  # unused by the operator
    tables = _build_tables(image_r, image_i, flow)
    csm_r = np.ascontiguousarray(csm_r, np.float32)
    csm_i = np.ascontiguousarray(csm_i, np.float32)
    in_maps = []
    for core in range(8):
        t, h = divmod(core, 2)
        sl = slice(h * S, (h + 1) * S)
        tbl, idx16, w4 = tables[t]
        in_maps.append({
            "csm_r": csm_r,
            "csm_i": csm_i,
            "kx": np.ascontiguousarray(traj[sl, 0, t], np.float32),
            "ky": np.ascontiguousarray(traj[sl, 1, t], np.float32),
            "tbl": np.ascontiguousarray(tbl),
            "idx": np.ascontiguousarray(idx16),
            "w4": np.ascontiguousarray(w4),
        })
    return in_maps


def combine_outputs(results):
    out = np.zeros((NCOIL, NS), np.complex64)
    for core, res in enumerate(results):
        t, h = divmod(core, 2)
        sl = slice(h * S, (h + 1) * S)
        out[:, sl] += res["out_r"].astype(np.complex64) + 1j * res["out_i"].astype(
            np.complex64)
    return out


def kernel(**inputs) -> np.ndarray:
    from concourse.bass_utils import run_bass_kernel_spmd

    nc = _get_nc()
    in_maps = make_in_maps(**inputs)
    res = run_bass_kernel_spmd(nc, in_maps, core_ids=list(range(8)))
    return combine_outputs(res.results)


# revision 18
# speedup vs baseline: 1.7255x; 1.0110x over previous
"""Batchelor GPU-NUFFT forward operator on 8 Trainium2 NeuronCores.

Math (per timepoint t):
    warped  = bilinear_warp(image, flow[..., t])
    coil    = csm * warped                                  [Nc,Nx,Ny]
    out_t[c,s] = sum_{x,y} coil[c,x,y] exp(-2pi i (kx_s (x-64) + ky_s (y-64)))
    out     = sum_t out_t                                   [Nc,NS] complex64

Sharding: 8 cores = 4 timepoints x 2 sample-halves (4096 samples each).
Host unshard: sum the 4 timepoint partials per half, concat halves.

Device pipeline (per core):
  * warp: host provides the bf16 corner table (DRAM, row (y0*128+x0) holds the
    4 bilinear corners of real+imag), int16 gather indices in the SWDGE
    wrapped layout, and the 4 bilinear weight planes. 16 dma_gather ops land
    the corners directly in [x, y] layout (slot i = y*128 + x); the combine
    and the coil pack run per 4-gather quarter in the gather shadow.
  * NUFFT: Khatri-Rao split y = yo*8 + yi. Per 512-sample chunk, 32
    accumulating bf16 matmuls build PSUM partials Pr = Re(sum coil e^{-iA}),
    Pi = Im(...) directly (stationary blocks Cr | Ci | -Ci make the +- signs
    accumulate in PSUM). The outer phase e^{-iB} is 4 elementwise products,
    folded to 8 coils by +-selector matmuls.
  * trig: phases are range-reduced with custom DVE ops (PHASE_WRAP fuses the
    a0 = kx*(x-64) - 64*ky wrap to one op; ADD_TT_WRAP fuses each chain step
    m2_yi = wrap(m2_parent + ky2^j) with a log-depth parent tree; ABS_SUB
    preps cos args as |m|-1/4 since the ACT Sin spline is only valid on
    [-pi, pi]). ky2/ky4 = wrap(2ky), wrap(4ky) come from the host. All four
    1024-wide trig batches are emitted ahead of the MM loop so the Scalar
    engine streams Sin evaluations while the gather runs.
"""

import sys

if "/opt/trn_rl_repo" not in sys.path:
    sys.path.insert(0, "/opt/trn_rl_repo")

import math

import numpy as np
import ml_dtypes

import concourse.bass as bass
import concourse.tile as tile
from concourse import bacc
from concourse import mybir
from concourse import dve_ops
from concourse.dve_spec import Spec, Src0, Src1, C0, C1, C2, Zero, maxx

P = 128
NX = 128
NCOIL = 8
NS = 8192
NT = 4
S = 4096           # samples per core (half of NS)
CH = 512           # samples per MM chunk (PSUM bank width)
NCHUNK = S // CH   # 8
BW = 1024          # trig batch width (2 chunks)
NBATCH = S // BW   # 4
YI = 8
YO = 16
NPIX = NX * NX
NGATH = 16
GIDX = NPIX // NGATH   # 1024 indices per gather
ELEM = 128             # bf16 elements per table row = 256 bytes
NABS_SC = 4            # yi < NABS_SC: cos-prep via scalar Abs; else DVE ABS_SUB
GP_OUTER = 0           # gpsimd cannot read PSUM: outer products stay on DVE

F32 = mybir.dt.float32
BF16 = mybir.dt.bfloat16
I16 = mybir.dt.int16
TWO_PI = float(2.0 * math.pi)
MAGIC = 12582912.0  # 1.5*2^23: (x + M) - M == round-to-nearest(x) for f32
ALU = mybir.AluOpType
ACTF = mybir.ActivationFunctionType


# ---------------- custom DVE ops ----------------
def _register_dve_op(name, spec):
    if name in dve_ops._SUB_OPCODE_FOR_NAME:
        for op in dve_ops.OPS:
            if op.name == name:
                return op
        raise RuntimeError(name)
    shas = {}
    for ver in ("v3", "v4"):
        uops = dve_ops.lower(spec, ver=ver)
        shas[ver] = dve_ops.DveOpSpec(
            name=name, opcode=0, uops=uops, rd1_en=dve_ops.has_src1(spec)
        ).sha(ver)
    op = dve_ops.DveOp(name, spec, subdim=False, uops_sha=shas)
    dve_ops.OPS.append(op)
    dve_ops._SUB_OPCODE_FOR_NAME[name] = (
        dve_ops._CUSTOM_DVE_ROW_BASE + len(dve_ops.OPS) - 1
    )
    dve_ops.CUSTOM_DVE_SPECS[name] = spec
    return op


def _wrap_np(v):
    return (v - np.round(v)).astype(np.float32)


# out = m - round(m), m = in0*s0 + in1*s1  (s0 may be a [P,1] AP)
_pw_m = Src0 * C0 + Src1 * C1
_pw_r = (_pw_m + C2) - C2
PHASE_WRAP = _register_dve_op(
    "PHASE_WRAP_ANT",
    Spec(
        body=_pw_m - _pw_r,
        reference=lambda in0, in1, s0, s1, imm2: (
            (in0 * s0 + in1 * s1)
            - (((in0 * s0 + in1 * s1) + imm2) - imm2)
        ).astype(np.float32),
    ),
)

# out = y - ((y > .5) - (y < -.5)), y = in0 + in1 : one-period wrap of a sum
_aw_y = Src0 + Src1
ADD_TT_WRAP = _register_dve_op(
    "ADD_TT_WRAP_ANT",
    Spec(
        body=_aw_y + C2 * ((_aw_y < (Zero - C1)) - (C1 < _aw_y)),
        reference=lambda in0, in1, s0, s1, imm2: (
            (in0 + in1)
            + imm2
            * (
                ((in0 + in1) < -s1).astype(np.float32)
                - ((in0 + in1) > s1).astype(np.float32)
            )
        ).astype(np.float32),
    ),
)

# out = |in0| + s0
ABS_SUB = _register_dve_op(
    "ABS_ADD_ANT",
    Spec(
        body=maxx(Src0, Zero - Src0) + C0,
        reference=lambda in0, in1, s0, s1, imm2: (np.abs(in0) + s0).astype(
            np.float32
        ),
    ),
)


def build_program(nc: bass.Bass, dbg: bool = False):
    def dbg_out(name, src_ap, shape, dtype=F32):
        if not dbg:
            return
        d = nc.dram_tensor("dbg_" + name, shape, dtype, kind="ExternalOutput").ap()
        nc.sync.dma_start(d[:], src_ap)

    csm_r = nc.dram_tensor("csm_r", [NCOIL, NX, NX], F32, kind="ExternalInput").ap()
    csm_i = nc.dram_tensor("csm_i", [NCOIL, NX, NX], F32, kind="ExternalInput").ap()
    kx_d = nc.dram_tensor("kx", [S], F32, kind="ExternalInput").ap()
    ky_d = nc.dram_tensor("ky", [S], F32, kind="ExternalInput").ap()
    tbl_d = nc.dram_tensor("tbl", [NPIX, ELEM], BF16, kind="ExternalInput").ap()
    idx_d = nc.dram_tensor("idx", [P, GIDX], I16, kind="ExternalInput").ap()
    w4_d = nc.dram_tensor("w4", [P, NX, 4], F32, kind="ExternalInput").ap()
    out_r = nc.dram_tensor("out_r", [NCOIL, S], F32, kind="ExternalOutput").ap()
    out_i = nc.dram_tensor("out_i", [NCOIL, S], F32, kind="ExternalOutput").ap()

    # ---------------- inline constants ----------------
    pvals = np.arange(P, dtype=np.float32)
    xc_d = nc.inline_tensor((pvals - 64.0).reshape(P, 1), name="c_xc").ap()
    yo8_d = nc.inline_tensor((8.0 * (np.arange(P) % 16)).astype(np.float32)
                             .reshape(P, 1), name="c_yo8").ap()
    half_pi_d = nc.inline_tensor(np.full((P, 1), math.pi / 2, np.float32),
                                 name="c_half_pi").ap()
    sel_np = (np.arange(P)[:, None] // 16 == np.arange(NCOIL)[None, :]).astype(
        np.float32)
    selpm_np = np.concatenate([sel_np, -sel_np], axis=1)  # [128, 16]: +sel | -sel
    selpm_d = nc.inline_tensor(selpm_np, name="c_selpm").ap()

    with tile.TileContext(nc) as tc, \
         tc.tile_pool(name="pp", bufs=1) as pp:

        # --- persistent constants / inputs ---
        idx16 = pp.tile([P, GIDX], I16)
        H = S // 2
        kxb = pp.tile([P, S], F32)
        nc.sync.dma_start(
            kxb[:, 0:H],
            kx_d[0:H].rearrange("(p s) -> p s", p=1).to_broadcast([P, H]))
        kyb = pp.tile([P, S], F32)
        nc.sync.dma_start(
            kyb[:, 0:H],
            ky_d[0:H].rearrange("(p s) -> p s", p=1).to_broadcast([P, H]))
        xc_col = pp.tile([P, 1], F32)
        nc.sync.dma_start(xc_col[:], xc_d[:])
        yo8 = pp.tile([P, 1], F32)
        nc.sync.dma_start(yo8[:], yo8_d[:])
        half_pi = pp.tile([P, 1], F32)
        nc.sync.dma_start(half_pi[:], half_pi_d[:])
        selpm32 = pp.tile([P, 2 * NCOIL], F32)
        nc.sync.dma_start(selpm32[:], selpm_d[:])
        selpm = pp.tile([P, 2 * NCOIL], BF16)
        nc.vector.tensor_copy(selpm[:], selpm32[:])

        # packed coil stationary: blocks [Cr | Ci | -Ci], col = c*16 + yo,
        # innermost yi so the pack writes contiguous 16B runs
        RA = pp.tile([P, 3, P, YI], BF16)

        # --- pools (gp innermost so it can close after the warp) ---
        lp_ctx = tc.tile_pool(name="loop", bufs=1)
        lp = lp_ctx.__enter__()
        kp_ctx = tc.tile_pool(name="kr", bufs=1)
        kp = kp_ctx.__enter__()
        gp_pool_ctx = tc.tile_pool(name="gp", bufs=1)
        gp = gp_pool_ctx.__enter__()
        g8p = gp.tile([P, NX, ELEM], BF16)
        w4sb = gp.tile([P, NX, 4], F32)
        nc.sync.dma_start(w4sb[:], w4_d[:])
        csm_r_sb = gp.tile([P, NCOIL, NX], F32)
        nc.sync.dma_start(csm_r_sb[:], csm_r.rearrange("c x y -> x c y"))
        csm_i_sb = gp.tile([P, NCOIL, NX], F32)
        nc.sync.dma_start(csm_i_sb[:], csm_i.rearrange("c x y -> x c y"))

        # idx16 is loaded LAST on the sync queue: HWDGE executes FIFO, so the
        # gathers (which depend on idx16) cannot start stealing SDMA slots
        # until every other input DMA has landed.
        nc.sync.dma_start(idx16[:], idx_d[:])
        nc.sync.dma_start(
            kxb[:, H:S],
            kx_d[H:S].rearrange("(p s) -> p s", p=1).to_broadcast([P, S - H]))
        nc.sync.dma_start(
            kyb[:, H:S],
            ky_d[H:S].rearrange("(p s) -> p s", p=1).to_broadcast([P, S - H]))
        gsems = [nc.alloc_semaphore(f"gath_sem{q}") for q in range(4)]
        for h in range(NGATH):
            nc.gpsimd.dma_gather(
                out_ap=g8p[:, h * 8:(h + 1) * 8, :],
                in_ap=tbl_d[:],
                idxs_ap=idx16[:, h * 64:(h + 1) * 64],
                num_idxs=GIDX,
                num_idxs_reg=GIDX,
                elem_size=ELEM,
                queue_num=h % 4,
            ).then_inc(gsems[h % 4], 16)

        # ---------------- trig batches (emitted in pieces) ----------------
        trig = {}

        def make_trig(b):
            cs = slice(b * BW, (b + 1) * BW)
            st = {"m2": {}, "kits": [], "krts": []}

            def emit_yi(yi):
                m2 = st["m2"]
                kyc = st["kyc"]
                if yi > 0:
                    t = lp.tile([P, BW], F32, tag="m2c", bufs=2)
                    nc.vector._custom_dve(ADD_TT_WRAP, out=t[:],
                                          in0=m2[yi - 1][:],
                                          in1=kyc, s1=0.5, imm2=1.0)
                    m2[yi] = t
                kit = kp.tile([P, BW], BF16, tag=f"kit{yi}", bufs=2)
                nc.scalar.activation(kit[:], m2[yi][:], ACTF.Sin, scale=-TWO_PI)
                krt = kp.tile([P, BW], BF16, tag=f"krt{yi}", bufs=2)
                if yi < NABS_SC:
                    mabs = lp.tile([P, BW], F32, tag="mabs", bufs=1)
                    nc.scalar.activation(mabs[:], m2[yi][:], ACTF.Abs)
                    nc.scalar.activation(krt[:], mabs[:], ACTF.Sin,
                                         scale=-TWO_PI, bias=half_pi[:, 0:1])
                else:
                    mk = lp.tile([P, BW], F32, tag="mk", bufs=2)
                    nc.vector._custom_dve(ABS_SUB, out=mk[:], in0=m2[yi][:],
                                          s0=-0.25)
                    nc.scalar.activation(krt[:], mk[:], ACTF.Sin, scale=-TWO_PI)
                st["kits"].append(kit)
                st["krts"].append(krt)

            def piece0():
                kxc = kxb[:, cs]
                kyc = st["kyc"] = kyb[:, cs]
                m2o = lp.tile([P, BW], F32, tag="m2o", bufs=1)
                nc.vector._custom_dve(PHASE_WRAP, out=m2o[:], in0=kyc,
                                      in1=kyc, s0=yo8[:, 0:1], s1=0.0,
                                      imm2=MAGIC)
                mok = lp.tile([P, BW], F32, tag="mok", bufs=1)
                nc.vector._custom_dve(ABS_SUB, out=mok[:], in0=m2o[:], s0=-0.25)
                aic = kp.tile([P, BW], BF16, tag="aic", bufs=2)
                nc.scalar.activation(aic[:], m2o[:], ACTF.Sin, scale=-TWO_PI)
                arc = kp.tile([P, BW], BF16, tag="arc", bufs=2)
                nc.scalar.activation(arc[:], mok[:], ACTF.Sin, scale=-TWO_PI)
                m2a = lp.tile([P, BW], F32, tag="m2a", bufs=1)
                nc.vector._custom_dve(PHASE_WRAP, out=m2a[:], in0=kxc,
                                      in1=kyc, s0=xc_col[:, 0:1], s1=-64.0,
                                      imm2=MAGIC)
                st["m2"][0] = m2a
                emit_yi(0)
                trig[b] = (st["kits"], st["krts"], arc, aic)

            return [piece0] + [lambda yi=yi: emit_yi(yi) for yi in range(1, YI)]

        # ---------------- warp eighth: combine + pack ----------------
        def emit_quarter(q):
            W = 16
            ys = slice(W * q, W * q + W)
            for h in (2 * q, 2 * q + 1):
                nc.vector.wait_ge(gsems[h % 4], 16 * (h // 4 + 1))
            t8r = gp.tile([P, W, 4], F32, tag="t8r", bufs=2)
            nc.vector.tensor_tensor(t8r[:], g8p[:, ys, 0:4], w4sb[:, ys, :],
                                    op=ALU.mult)
            warped_r = gp.tile([P, W], F32, tag="wr", bufs=2)
            nc.vector.reduce_sum(warped_r[:], t8r[:], axis=mybir.AxisListType.X)
            t8i = gp.tile([P, W, 4], F32, tag="t8i", bufs=2)
            nc.vector.tensor_tensor(t8i[:], g8p[:, ys, 4:8], w4sb[:, ys, :],
                                    op=ALU.mult)
            warped_i = gp.tile([P, W], F32, tag="wi", bufs=2)
            nc.vector.reduce_sum(warped_i[:], t8i[:], axis=mybir.AxisListType.X)

            wr_b = warped_r[:].rearrange("p (c y) -> p c y", c=1).to_broadcast(
                [P, NCOIL, W])
            wi_b = warped_i[:].rearrange("p (c y) -> p c y", c=1).to_broadcast(
                [P, NCOIL, W])
            csr = csm_r_sb[:, :, ys]
            csi = csm_i_sb[:, :, ys]

            # RA views for this eighth: [p, c, yo(2), yi(8)], contiguous yi
            ra5 = RA[:].rearrange("p b (c yo) yi -> p b c yo yi", c=NCOIL)
            NYO = W // YI
            cr_v = ra5[:, 0, :, NYO * q:NYO * q + NYO, :]
            ci_v = ra5[:, 1, :, NYO * q:NYO * q + NYO, :]
            cin_v = ra5[:, 2, :, NYO * q:NYO * q + NYO, :]

            def as4(t):
                return t.rearrange("p c (yo yi) -> p c yo yi", yi=YI)

            tt1 = gp.tile([P, NCOIL, W], F32, tag="tt1", bufs=2)
            nc.vector.tensor_tensor(tt1[:], csr, wr_b, op=ALU.mult)
            tt2 = gp.tile([P, NCOIL, W], F32, tag="tt2", bufs=2)
            nc.vector.tensor_tensor(tt2[:], csi, wi_b, op=ALU.mult)
            nc.vector.tensor_tensor(cr_v, as4(tt1[:]), as4(tt2[:]),
                                    op=ALU.subtract)
            tt3 = gp.tile([P, NCOIL, W], F32, tag="tt1", bufs=2)
            nc.vector.tensor_tensor(tt3[:], csr, wi_b, op=ALU.mult)
            tt4 = gp.tile([P, NCOIL, W], F32, tag="tt2", bufs=2)
            nc.vector.tensor_tensor(tt4[:], csi, wr_b, op=ALU.mult)
            cit = gp.tile([P, NCOIL, W], F32, tag="cit", bufs=2)
            nc.vector.tensor_tensor(cit[:], tt3[:], tt4[:], op=ALU.add)
            nc.vector.tensor_copy(ci_v, as4(cit[:]))
            nc.vector.tensor_scalar(cin_v, as4(cit[:]), -1.0, None, op0=ALU.mult)

        # ---------------- MM chunk ----------------
        ps_ctx = tc.tile_pool(name="ps", bufs=1, space="PSUM")
        ps = ps_ctx.__enter__()
        pso_ctx = tc.tile_pool(name="pso", bufs=1, space="PSUM")
        pso = pso_ctx.__enter__()

        live = {}

        def emit_mains(ch):
            b, half = divmod(ch, 2)
            sl = slice(half * CH, (half + 1) * CH)
            kits, krts, arc, aic = trig[b]
            Pr = ps.tile([P, CH], F32, tag="Pr", bufs=3)
            Pi = ps.tile([P, CH], F32, tag="Pi", bufs=3)
            for yi in range(YI):
                st, sp = (yi == 0), (yi == YI - 1)
                krt_s = krts[yi][:, sl]
                kit_s = kits[yi][:, sl]
                nc.tensor.matmul(Pr[:], RA[:, 0, :, yi], krt_s,
                                 start=st, stop=False)
                nc.tensor.matmul(Pi[:], RA[:, 0, :, yi], kit_s,
                                 start=st, stop=False)
                nc.tensor.matmul(Pr[:], RA[:, 2, :, yi], kit_s,
                                 start=False, stop=sp)
                nc.tensor.matmul(Pi[:], RA[:, 1, :, yi], krt_s,
                                 start=False, stop=sp)
            live[ch] = (Pr, Pi, arc, aic, sl)

        def emit_post(ch):
            c0 = ch * CH
            Pr, Pi, arc, aic, sl = live.pop(ch)
            q1 = lp.tile([P, CH], BF16, tag="q1", bufs=2)
            nc.vector.tensor_tensor(q1[:], Pr[:], arc[:, sl], op=ALU.mult)
            q2 = lp.tile([P, CH], BF16, tag="q2", bufs=2)
            nc.vector.tensor_tensor(q2[:], Pi[:], aic[:, sl], op=ALU.mult)
            eng3 = nc.gpsimd if GP_OUTER >= 1 else nc.vector
            eng4 = nc.gpsimd if GP_OUTER >= 2 else nc.vector
            q3 = lp.tile([P, CH], BF16, tag="q3", bufs=2)
            eng3.tensor_tensor(q3[:], Pi[:], arc[:, sl], op=ALU.mult)
            q4 = lp.tile([P, CH], BF16, tag="q4", bufs=2)
            eng4.tensor_tensor(q4[:], Pr[:], aic[:, sl], op=ALU.mult)

            SP, SM = selpm[:, 0:NCOIL], selpm[:, NCOIL:2 * NCOIL]
            po = pso.tile([32 + NCOIL, CH], F32, tag="po", bufs=2)
            nc.tensor.matmul(po[0:NCOIL], SP, q1[:], start=True, stop=False)
            nc.tensor.matmul(po[0:NCOIL], SM, q2[:], start=False, stop=True)
            nc.tensor.matmul(po[32:32 + NCOIL], SP, q3[:], start=True,
                             stop=False)
            nc.tensor.matmul(po[32:32 + NCOIL], SP, q4[:], start=False,
                             stop=True)
            ost = lp.tile([32 + NCOIL, CH], F32, tag="ost", bufs=2)
            nc.scalar.copy(ost[:], po[:])
            nc.sync.dma_start(out_r[:, c0:c0 + CH], ost[0:NCOIL])
            nc.sync.dma_start(out_i[:, c0:c0 + CH], ost[32:32 + NCOIL])

        # ---------------- emission schedule ----------------
        for p in make_trig(0):
            p()
        for q in range(8):
            emit_quarter(q)
        dbg_out("RA", RA[:].rearrange("p b c yi -> p (b c yi)"), [P, YI * 3 * P],
                BF16)
        gp_pool_ctx.__exit__(None, None, None)
        for p in make_trig(1):
            p()

        # interleave trig batches 2/3 into the MM loop so the DVE queue never
        # blocks the selector matmuls: b2 pieces land after posts 1-3, b3
        # after posts 3-5.
        t2 = make_trig(2)
        t3 = make_trig(3)
        pieces = {0: t2[0:3], 1: t2[3:6], 2: t2[6:8] + t3[0:1],
                  3: t3[1:4], 4: t3[4:8]}

        for ch in range(NCHUNK):
            emit_mains(ch)
            if ch > 0:
                emit_post(ch - 1)
                for p in pieces.get(ch - 1, []):
                    p()
        emit_post(NCHUNK - 1)

        pso_ctx.__exit__(None, None, None)
        ps_ctx.__exit__(None, None, None)
        kp_ctx.__exit__(None, None, None)
        lp_ctx.__exit__(None, None, None)


_COMPILED = {}


def _get_nc(dbg: bool = False):
    key = ("nc", dbg)
    if key not in _COMPILED:
        nc = bacc.Bacc("TRN2", debug=False, num_swdge_queues=4)
        build_program(nc, dbg=dbg)
        nc.compile()
        _COMPILED[key] = nc
    return _COMPILED[key]


# slot g = 16*j + (p%16)  <->  output pixel (x = g%128, y = g//128);
# gather h covers slots [1024h, 1024(h+1)) -> partitions x, columns y.
_Jg = np.arange(GIDX)[None, :]
_Pg = np.arange(P)[:, None]
_G = 16 * _Jg + (_Pg % 16)            # [128, 1024]
_XG = (_G % 128).astype(np.int64)
_YG = (_G // 128).astype(np.int64)
_BF16 = ml_dtypes.bfloat16


def _build_tables(image_r, image_i, flow):
    """Per-timepoint: corner table (bf16, row y0*128+x0), idx16, weights."""
    ir = np.ascontiguousarray(image_r, np.float32)
    ii = np.ascontiguousarray(image_i, np.float32)
    irT, iiT = ir.T, ii.T                     # [y, x]
    y1 = np.minimum(np.arange(NX) + 1, NX - 1)
    x1 = np.minimum(np.arange(NX) + 1, NX - 1)
    tables = []
    for t in range(NT):
        f0 = np.asarray(flow[:, :, 0, t], np.float32)
        f1 = np.asarray(flow[:, :, 1, t], np.float32)
        # float32 math mirrors the jax reference exactly
        xg = np.arange(NX, dtype=np.float32)[:, None]
        yg = np.arange(NX, dtype=np.float32)[None, :]
        cx = np.clip(xg + f0, np.float32(0.0), np.float32(NX - 1))
        cy = np.clip(yg + f1, np.float32(0.0), np.float32(NX - 1))
        x0 = np.floor(cx)
        y0 = np.floor(cy)
        wx = (cx - x0).astype(np.float32)     # [x, y]
        wy = (cy - y0).astype(np.float32)
        w4 = np.stack([(1 - wx) * (1 - wy), (1 - wx) * wy,
                       wx * (1 - wy), wx * wy], axis=-1).astype(np.float32)
        x0i = x0.astype(np.int64)
        y0i = y0.astype(np.int64)
        idxv = (y0i * NX + x0i).astype(np.int16)      # [x, y]
        idx16 = idxv[_XG, _YG]                        # wrapped gather layout

        tbl = np.zeros((NX, NX, ELEM), dtype=_BF16)
        tbl[:, :, 0] = irT
        tbl[:, :, 1] = irT[y1, :]
        tbl[:, :, 2] = irT[:, x1]
        tbl[:, :, 3] = irT[y1][:, x1]
        tbl[:, :, 4] = iiT
        tbl[:, :, 5] = iiT[y1, :]
        tbl[:, :, 6] = iiT[:, x1]
        tbl[:, :, 7] = iiT[y1][:, x1]
        tables.append((tbl.reshape(NPIX, ELEM), idx16, w4))
    return tables


def make_in_maps(image_r, image_i, csm_r, csm_i, traj, dcf, flow):
    del dcf  # unused by the operator
    tables = _build_tables(image_r, image_i, flow)
    csm_r = np.ascontiguousarray(csm_r, np.float32)
    csm_i = np.ascontiguousarray(csm_i, np.float32)
    in_maps = []
    for core in range(8):
        t, h = divmod(core, 2)
        sl = slice(h * S, (h + 1) * S)
        tbl, idx16, w4 = tables[t]
        in_maps.append({
            "csm_r": csm_r,
            "csm_i": csm_i,
            "kx": np.ascontiguousarray(traj[sl, 0, t], np.float32),
            "ky": np.ascontiguousarray(traj[sl, 1, t], np.float32),
            "tbl": np.ascontiguousarray(tbl),
            "idx": np.ascontiguousarray(idx16),
            "w4": np.ascontiguousarray(w4),
        })
    return in_maps


def combine_outputs(results):
    out = np.zeros((NCOIL, NS), np.complex64)
    for core, res in enumerate(results):
        t, h = divmod(core, 2)
        sl = slice(h * S, (h + 1) * S)
        out[:, sl] += res["out_r"].astype(np.complex64) + 1j * res["out_i"].astype(
            np.complex64)
    return out


def kernel(**inputs) -> np.ndarray:
    from concourse.bass_utils import run_bass_kernel_spmd

    nc = _get_nc()
    in_maps = make_in_maps(**inputs)
    res = run_bass_kernel_spmd(nc, in_maps, core_ids=list(range(8)))
    return combine_outputs(res.results)
